# revision 29
# baseline (speedup 1.0000x reference)
"""Trainium2 Bass kernel for nn_Apply_on_single_area.

Computes, per supervoxel area b:
    loss[b] = sum_{i,j} eroded(mc)[i,j] * em[i,j]
where mc = mask_combined[..., mask_index] with last row/col zeroed and
eroded = E(a1)*E(a2), E(a) = 2a - a^2, a1/a2 = products with the next
element along each spatial axis (zero-padded).

Key simplifications / design (HW-measured on TRN2):
- differentiable_or_simple(a,b) = a*b + (1-a)*a + (1-b)*a = 2a - a^2:
  the b-terms cancel, so only forward-neighbor products a1, a2 matter.
- Only rows/cols 0..29 can contribute (row/col 31 are zeroed, which
  forces e=0 on row 30 / col 30 as well), so the host packs mc at
  31-wide stride (31x31=961) and em as 30x31=930 with col 30 zeroed
  ("p31"): flat shifts become +31/+1 and every stream shrinks ~6%.
- Pure data parallel: B=10000 split 1250/core over 8 cores, padded to
  1280 = 128 partitions x 10 areas, partition-major so every DMA is
  contiguous per partition.
- bf16 compute, f32 accumulation; em is stored fp8e4m3 in DRAM and
  cast to bf16 *by the DMA engine* during the load (SWDGE dtype-cast,
  "ef8"): -24% HBM bytes at zero compute cost, rel err 4.6e-3 vs the
  2e-2 gate (exact-sim'd on the real data before adoption).
- DVE is the bottleneck engine; the "cg" design cuts its cycles 20%+
  below the previous ANT_EE2 config by fusing the shift-product INTO
  the e() evaluation with a runtime-registered custom DVE op ANT_EG2X:
  g = (Src0*Src1)*(2 - Src0*Src1) = e(t), 3 ALU stages, with a
  HAND-AUTHORED 2X_1PORT uop (2 elems/cycle; the body replayed at
  stages 3-5 for the odd element, even result parked in delay lane 0,
  out LO<-DELAY_0 / HI<-ALU_OUT - mirrors the stock 3-ALU op at
  table_ptr 104; perf_max=1 set on the instruction enables the mode).
  Verified bit-level on HW: rel err identical to the stock path.
  Per area the DVE does g1 = EG(m0,m31), g2 = EG(m0,m1) (2X custom),
  p = g1*g2 and w = p*em (stock 2x tt); ACT does the per-area
  Copy+accum reductions (measured off the critical path).
- One A=10 supertile per pass ("a10", w multiplied in place into the p
  tile "wip" so mid/ld pools at 2/2 "b22" fit SBUF): halves the DVE
  instruction-dispatch overhead vs a5; chained reps still pipeline
  through the pools. GPSIMD is only the SWDGE cast-DMA queue; gpsimd
  *compute* measured ~10us slower (Multiply impl efficiency 0.42).
- HW-measured ablations (noisy axon timing, pairwise-interleaved
  medians): ANT_EE2 1x pass ~5us marginal, each t12 tt ~3-6us, ACT
  reduces ~0 (slack), DMA-only floor ~10.4us at the 3.65MB/core/pass
  stream. cg measured 18.1us vs cp 24.2us in the same window (-25%),
  and cg-a10-wip beat cg-a5 in two independent windows (-0.9/-1.7us);
  the harness-scale equivalent of cp was 14.5us. Tested and rejected
  at this operating point (all within noise of the default, so the
  kernel sits at the DVE roofline for its op structure): mc-fp8 "f8"
  (1.18e-2, thin gate margin), mc-uint8 fixed-point "mq8" via
  ANT_EGQ2X (6.1e-3, works but no speed win - loop is not DMA-bound),
  separate g1/g2 tiles "psep", ACT e12 offload "eahN" (ACT saturates),
  DMA-CCE em-multiply (compiler rejects cast+mult), gpsimd compute.
  Stock instructions have no perf_max field, so the stock tt ops
  cannot be forced into the 4X_2P table slots.
Fallback chain: cg-a5, then cp (ANT_EE2, harness-proven), then a
stock-op-only variant.
"""

import numpy as np

import jax
from jax.experimental.shard_map import shard_map
from jax.sharding import Mesh, NamedSharding, PartitionSpec

import concourse.bass as bass
import concourse.bacc as bacc
import concourse.mybir as mybir
import concourse.tile as tile
from concourse import bass2jax


def _register_ee2():
    """Custom DVE op: out = e(Src0)*e(Src1), e(t) = 1-(1-t)^2.

    Fuses the ACT Square (u12), DVE tensor_scalar (e12) and DVE
    tensor_tensor (p) into one 1x-rate DVE pass: same DVE cycles as the
    e12+p pair it replaces, but removes u12 (2/3 of ACT work) entirely.
    Registered at import so the op's table rows ship in our NEFF; sha is
    pinned from a fresh lower() (semantics verified against reference)."""
    from concourse import dve_ops
    from concourse.dve_spec import Spec, Src0, Src1, One, lower
    from concourse.dve_uop import DveOpSpec

    if any(op.name == "ANT_EE2" for op in dve_ops.OPS):
        return next(op for op in dve_ops.OPS if op.name == "ANT_EE2")

    a1 = One - Src0
    u1 = a1 * a1
    a2 = One - Src1
    u2 = a2 * a2
    spec = Spec(
        body=(One - u1) * (One - u2),
        reference=lambda in0, in1: (1 - (1 - in0) ** 2) * (1 - (1 - in1) ** 2),
    )
    tmp = dve_ops.DveOp("ANT_EE2", spec, subdim=False, uops_sha={})
    dve_ops.OPS.append(tmp)
    dve_ops._SUB_OPCODE_FOR_NAME["ANT_EE2"] = (
        dve_ops._CUSTOM_DVE_ROW_BASE + len(dve_ops.OPS) - 1
    )
    opcode = dve_ops.get_dve_sub_opcode("ANT_EE2")
    shas = {}
    for ver in ("v3", "v4"):
        ds = DveOpSpec(
            name="ANT_EE2", opcode=opcode, uops=lower(spec, ver=ver), rd1_en=True
        )
        shas[ver] = ds.sha(ver)
    final = dve_ops.DveOp("ANT_EE2", spec, subdim=False, uops_sha=shas)
    dve_ops.OPS[-1] = final
    return final


ANT_EE2 = _register_ee2()


def _register_eg2x():
    """Custom DVE op ANT_EG2X: out = g(Src0*Src1), g(t) = t*(s0-t), WITH a
    hand-authored 2X_1PORT uop (2 elems/cycle for bf16 packed operands).

    g(t) with s0=2 equals e(t) = 1-(1-t)^2, so e(t1) = EG(m0, mW) fuses the
    t-product INTO the e() evaluation. Body is 3 ALUs (mult, sub, mult) ->
    the 2X variant replays it at stages 3-5 for the odd element (inputs via
    SRC_0_HI/SRC_1_HI delay lanes, even result parked in d0, out LO<-DELAY_0
    HI<-ALU_OUT), mirroring the stock 3-ALU op at table_ptr 104. perf_max=1
    on the instruction caps the engine at 2X_1P (2X_2P/4X slots hold the
    same uop as don't-care fallbacks but are unreachable)."""
    from concourse import dve_ops
    from concourse.dve_spec import Spec, Src0, Src1, C0, lower
    from concourse.dve_uop import (
        AluInp,
        AluOp,
        DveOpSpec,
        InpSel,
        OutPath,
        OutSel,
        Trigger,
        UopConfig,
        UopDpConfig,
    )

    NAME = "ANT_EG2X"
    if any(op.name == NAME for op in dve_ops.OPS):
        return next(op for op in dve_ops.OPS if op.name == NAME)

    t = Src0 * Src1
    spec = Spec(
        body=t * (C0 - t),
        reference=lambda in0, in1, s0: (in0 * in1) * (s0 - in0 * in1),
    )

    def PD(i):
        return AluInp(AluInp.PREV_DELAY_0 + i)

    PASS = 5  # DelayInp.PREV_DELAY
    CAP = 0  # DelayInp.PREV_ALU_OUT

    def blk(op=AluOp.BYPASS, s0=AluInp.PREV_ALU_OUT, s1=AluInp.PREV_ALU_OUT,
            d=(), cap=()):
        delay = [5] * 7
        delay_enable = [0] * 7
        for i in d:
            delay[i] = PASS
            delay_enable[i] = 1
        for i in cap:
            delay[i] = CAP
            delay_enable[i] = 1
        from concourse.dve_uop import DelayInp
        return UopDpConfig(
            op=op, alu_src0=s0, alu_src1=s1,
            delay=[DelayInp(x) for x in delay],
            alu_out_enable=1, delay_enable=delay_enable,
        )

    M, S = AluOp.MULTIPLY, AluOp.SUBTRACT
    uop2x = UopConfig(
        inp=[InpSel.ZERO, InpSel.SRC_0, InpSel.SRC_1, InpSel.CONST_0,
             InpSel.SRC_0_HI, InpSel.SRC_1_HI, InpSel.ZERO, InpSel.ZERO],
        inp_enable=[0, 1, 1, 1, 1, 1, 0, 0],
        out={OutPath.WR0_LO: OutSel.DELAY_0, OutPath.WR0_HI: OutSel.ALU_OUT,
             OutPath.WR1_LO: OutSel.ALU_OUT, OutPath.WR1_HI: OutSel.ALU_OUT},
        out_enable={OutPath.WR0_LO: 1, OutPath.WR0_HI: 1,
                    OutPath.WR1_LO: 0, OutPath.WR1_HI: 0},
        require_inp0=1, require_inp1=1,
        trigger=(Trigger.SRC_TENSOR_DONE, Trigger.NONE, Trigger.NONE),
        next_uop=(0, 0, 0), repeat_count=0,
        datapath_config=[
            # stages 0-2: even element (same as REGULAR), B inputs ride d3,d4
            blk(M, PD(0), PD(1), d=(1, 2, 3, 4)),          # t_A = s0*s1
            blk(S, PD(2), AluInp.PREV_ALU_OUT, d=(2, 3, 4), cap=(0,)),  # C0-t_A; d0<-t_A
            blk(M, PD(0), AluInp.PREV_ALU_OUT, d=(2, 3, 4)),  # g_A = t_A*(C0-t_A)
            # stages 3-5: odd element; g_A parked in d0
            blk(M, PD(3), PD(4), d=(2,), cap=(0,)),          # t_B; d0<-g_A
            blk(S, PD(2), AluInp.PREV_ALU_OUT, d=(0,), cap=(1,)),  # C0-t_B; d1<-t_B
            blk(M, PD(1), AluInp.PREV_ALU_OUT, d=(0,)),       # g_B
            blk(d=(0,)),                                        # pass g_B + d0
            blk(d=(0,)),
        ],
    )
    uop2x.validate("v3")

    class DveOp2x:
        name = NAME
        subdim = False

        def __init__(self):
            self.spec = spec
            self._cache = {}

        def compile(self, ver):
            if ver in self._cache:
                return self._cache[ver]
            s = DveOpSpec(
                name=NAME,
                opcode=dve_ops.get_dve_sub_opcode(NAME),
                uops=lower(spec, ver=ver),
                rd1_en=True,
                uops_2x=[uop2x] if ver == "v3" else None,
            )
            self._cache[ver] = s
            return s

        def validate(self, ver):
            return self.compile(ver).validate(ver)

    dve_ops._SUB_OPCODE_FOR_NAME[NAME] = (
        dve_ops._CUSTOM_DVE_ROW_BASE + len(dve_ops.OPS)
    )
    op = DveOp2x()
    dve_ops.OPS.append(op)
    return op


ANT_EG2X = _register_eg2x()


def _register_egq2x():
    """ANT_EGQ2X: out = g(Src0*Src1*s1), g(u) = u*(s0-u), 4 ALUs, with a
    hand-authored 2X_1PORT uop (8 stages exactly). For uint8 fixed-point mc:
    m = q/255 -> s1 = 1/255^2, s0 = 2 gives g = e(m0*m1) with ~20x lower
    RMS quantization error than fp8e4m3 at the same 1 byte/elem."""
    from concourse import dve_ops
    from concourse.dve_spec import Spec, Src0, Src1, C0, C1, lower
    from concourse.dve_uop import (
        AluInp, AluOp, DelayInp, DveOpSpec, InpSel, OutPath, OutSel,
        Trigger, UopConfig, UopDpConfig,
    )

    NAME = "ANT_EGQ2X"
    if any(op.name == NAME for op in dve_ops.OPS):
        return next(op for op in dve_ops.OPS if op.name == NAME)

    t = Src0 * Src1
    u = t * C1
    spec = Spec(
        body=u * (C0 - u),
        reference=lambda in0, in1, s0, s1: (in0 * in1 * s1)
        * (s0 - in0 * in1 * s1),
    )

    def PD(i):
        return AluInp(AluInp.PREV_DELAY_0 + i)

    def blk(op=AluOp.BYPASS, s0=AluInp.PREV_ALU_OUT, s1=AluInp.PREV_ALU_OUT,
            d=(), cap=()):
        delay = [DelayInp.PREV_DELAY] * 7
        delay_enable = [0] * 7
        for i in d:
            delay_enable[i] = 1
        for i in cap:
            delay[i] = DelayInp.PREV_ALU_OUT
            delay_enable[i] = 1
        return UopDpConfig(op=op, alu_src0=s0, alu_src1=s1, delay=delay,
                           alu_out_enable=1, delay_enable=delay_enable)

    M, S = AluOp.MULTIPLY, AluOp.SUBTRACT
    # lanes: d0=s0 d1=s1 d2=C1 d3=C0 d4=s0_HI d5=s1_HI (matches lower()'s
    # REGULAR lane plan extended with the odd element)
    uop2x = UopConfig(
        inp=[InpSel.ZERO, InpSel.SRC_0, InpSel.SRC_1, InpSel.CONST_1,
             InpSel.CONST_0, InpSel.SRC_0_HI, InpSel.SRC_1_HI, InpSel.ZERO],
        inp_enable=[0, 1, 1, 1, 1, 1, 1, 0],
        out={OutPath.WR0_LO: OutSel.DELAY_0, OutPath.WR0_HI: OutSel.ALU_OUT,
             OutPath.WR1_LO: OutSel.ALU_OUT, OutPath.WR1_HI: OutSel.ALU_OUT},
        out_enable={OutPath.WR0_LO: 1, OutPath.WR0_HI: 1,
                    OutPath.WR1_LO: 0, OutPath.WR1_HI: 0},
        require_inp0=1, require_inp1=1,
        trigger=(Trigger.SRC_TENSOR_DONE, Trigger.NONE, Trigger.NONE),
        next_uop=(0, 0, 0), repeat_count=0,
        datapath_config=[
            blk(M, PD(0), PD(1), d=(2, 3, 4, 5)),           # t_A
            blk(M, AluInp.PREV_ALU_OUT, PD(2), d=(2, 3, 4, 5)),  # u_A = t_A*C1
            blk(S, PD(3), AluInp.PREV_ALU_OUT, d=(2, 3, 4, 5), cap=(0,)),  # C0-u_A; d0<-u_A
            blk(M, PD(0), AluInp.PREV_ALU_OUT, d=(2, 3, 4, 5)),  # g_A
            blk(M, PD(4), PD(5), d=(2, 3), cap=(0,)),       # t_B; d0<-g_A
            blk(M, AluInp.PREV_ALU_OUT, PD(2), d=(0, 3)),   # u_B
            blk(S, PD(3), AluInp.PREV_ALU_OUT, d=(0,), cap=(1,)),  # C0-u_B; d1<-u_B
            blk(M, PD(1), AluInp.PREV_ALU_OUT, d=(0,)),     # g_B
        ],
    )
    uop2x.validate("v3")

    class DveOpQ2x:
        name = NAME
        subdim = False

        def __init__(self):
            self.spec = spec
            self._cache = {}

        def compile(self, ver):
            if ver in self._cache:
                return self._cache[ver]
            s = DveOpSpec(
                name=NAME,
                opcode=dve_ops.get_dve_sub_opcode(NAME),
                uops=lower(spec, ver=ver),
                rd1_en=True,
                uops_2x=[uop2x] if ver == "v3" else None,
            )
            self._cache[ver] = s
            return s

        def validate(self, ver):
            return self.compile(ver).validate(ver)

    dve_ops._SUB_OPCODE_FOR_NAME[NAME] = (
        dve_ops._CUSTOM_DVE_ROW_BASE + len(dve_ops.OPS)
    )
    op = DveOpQ2x()
    dve_ops.OPS.append(op)
    return op


ANT_EGQ2X = _register_egq2x()

N_CORES = 8
B_TOTAL = 10000
SHARD = B_TOTAL // N_CORES  # 1250
C_PER_P = 10  # areas per partition (after padding shard to 1280)
SHARD_PAD = 128 * C_PER_P
AREA = 1024  # 32*32
W = 32
NV = AREA - W  # 992 valid flat positions (rows 0..30)

DEFAULT_VARIANT = "v10-k0-b22-flat-p31-cg-a10-ef8-wip"

F32 = mybir.dt.float32
BF16 = mybir.dt.bfloat16

_NC_CACHE = {}


def _supertiles(shard: int, A: int):
    """Split `shard` areas into supertiles (base, P, a) with a area-slots of
    P partitions each. Area index = base + 128*j + p for slot j, partition p."""
    out = []
    base = 0
    while shard - base >= 128 * A:
        out.append((base, 128, A))
        base += 128 * A
    while shard - base >= 128:
        out.append((base, 128, 1))
        base += 128
    if shard > base:
        out.append((base, shard - base, 1))
        base = shard
    return out


def _build(shard: int, inner_reps: int = 1, A: int = 2, variant: str | None = None) -> bass.Bass:
    if variant is None:
        variant = DEFAULT_VARIANT
    """Per-core SPMD graph: mc [1280,1024] bf16 (edges pre-zeroed, rows
    1250..1279 zero-padded), em [1280,992] bf16 -> out [1280] f32.

    Partition-major layout: area = p*C_PER_P + t, so every DMA is
    contiguous per partition (loads 2-4 KB lines, store one 40 B line).

    Math: loss = sum_k e(t1)*e(t2)*em with e(t) = t*(2-t) = 1-(1-t)^2,
    t1[k]=m[k]*m[k+32], t2[k]=m[k]*m[k+1] over k in [0,992).

    Two-engine split (HW-measured): DVE t1/t2 (same-tensor shifted tt),
    e=1-u (ts), p=e1*e2, w=p*em (tt); ACT squares u=(1-t)^2 and the
    final Copy+accum reduction per area. Lag-pipelined emission."""
    assert shard == SHARD_PAD, shard
    C = C_PER_P
    nc = bacc.Bacc("TRN2", target_bir_lowering=False, debug=False)

    # t9: positions k in [960,992) have t1 = m[k]*m[k+32] = 0 exactly (row 31
    # is zeroed) so e1 = 0 and they contribute nothing; skip loading/computing
    # them. mc only needs k in [0,992) (m[k+32] max index 991).
    # p31: host packs rows at stride 31 (dropping the zeroed col 31): mc is
    # 31x31=961, em is 30x31=930 with col 30 zeroed (kills the row-wrap
    # garbage at j=30). Stream is 930 elems/area, shifts +31/+1.
    if "p31" in variant:
        NVv, MCW, Wv = 930, 961, 31
        MC_DECL, EM_DECL = 961, 930
    else:
        NVv = 960 if "t9" in variant else NV
        MCW = 992 if "t9" in variant else AREA
        Wv = W
        MC_DECL, EM_DECL = AREA, NV

    # f8: inputs stored fp8e4m3 in DRAM, cast to bf16 by the DMA engine
    # during the load (SWDGE dtype-cast path) - halves HBM bytes at zero
    # compute cost. ef8: em only (tighter accuracy margin keeps mc bf16).
    F8 = mybir.dt.float8e4
    mc_f8 = "-f8" in variant
    mc_q8 = "-mq8" in variant
    em_f8 = "-f8" in variant or "-ef8" in variant
    mc_dt = mybir.dt.uint8 if mc_q8 else (F8 if mc_f8 else BF16)
    mc_d = nc.declare_dram_parameter(
        "mc", [shard, MC_DECL], mc_dt, isOutput=False
    )
    em_d = nc.declare_dram_parameter(
        "em", [shard, EM_DECL], F8 if em_f8 else BF16, isOutput=False
    )
    out_d = nc.declare_dram_parameter("out", [shard], F32, isOutput=True)

    if "-a10" in variant:
        A = 10
    elif "-a5" in variant:
        A = 5
    n_super = C // A
    AL = mybir.AluOpType
    AF = mybir.ActivationFunctionType
    mc_v = mc_d.ap().rearrange("(p c) k -> p c k", c=C)
    em_v = em_d.ap().rearrange("(p c) k -> p c k", c=C)

    reuse = "reuse" in variant or "bufs6" in variant
    mid_bufs = 2 if ("b22" in variant or "b23" in variant or "b24" in variant) else (6 if "bufs6" in variant else (3 if ("mix" in variant or "b33" in variant) else (5 if "b53" in variant else 4)))
    lag_c = 1 if "lag1" in variant else (3 if "lag3" in variant else 2)
    ld_bufs = 4 if "b24" in variant else 3 if "b23" in variant else 2 if ("ldb2" in variant or "b22" in variant) else (6 if "ldb6" in variant else (3 if ("b53" in variant or "b33" in variant) else 4))
    # eaN: supertiles s < N compute e12 = 1-u12 on ACT (Copy scale=-1 bias=1)
    # instead of DVE tensor_scalar, shifting ~992c/supertile off DVE.
    ea_n = 0
    if "-ea" in variant and "-eah" not in variant:
        ea_n = int(variant.split("-ea")[1][0])
    with tile.TileContext(nc) as tc:
        with (
            tc.tile_pool(name="ld", bufs=ld_bufs) as ld,
            tc.tile_pool(name="mid", bufs=mid_bufs) as mid,
            tc.tile_pool(name="res", bufs=4) as resp,
            tc.tile_pool(name="stat", bufs=1) as statp,
            tc.tile_pool(name="ps", bufs=2, space="PSUM") as psp,
        ):
          if "mix" in variant and inner_reps:
            stat = {}
            for nm, shp in [("sm", [128, A, AREA]), ("se", [128, A, NV]),
                            ("st", [128, 2, A, NV]), ("su", [128, 2, A, NV]),
                            ("sе12", [128, 2, A, NV]), ("sp", [128, A, NV]),
                            ("sw", [128, A, NV])]:
                t = statp.tile(shp, BF16, tag="stat_" + nm)
                nc.vector.memset(t[:], 0.25)
                stat[nm] = t
          for _rr in range(inner_reps):
            res_t = resp.tile([128, C], F32, tag="res")
            stage_state = {}
            if "mix" in variant:
                for s in range(n_super):
                    c0 = s * A
                    m = ld.tile([128, A, AREA], BF16, tag="m")
                    nc.sync.dma_start(out=m[:], in_=mc_v[:, c0 : c0 + A, :])
                    e = ld.tile([128, A, NV], BF16, tag="e")
                    nc.sync.dma_start(out=e[:], in_=em_v[:, c0 : c0 + A, :])
                    t12 = mid.tile([128, 2, A, NV], BF16, tag="t12")
                    sm = stat["sm"]
                    nc.vector.tensor_tensor(t12[:, 0], sm[:, :, 0:NV], sm[:, :, W:AREA], AL.mult)
                    nc.vector.tensor_tensor(t12[:, 1], sm[:, :, 0:NV], sm[:, :, 1 : 1 + NV], AL.mult)
                    u12 = mid.tile([128, 2, A, NV], BF16, tag="u12")
                    nc.scalar.activation(u12[:], stat["st"][:], AF.Square, bias=1.0, scale=-1.0)
                    e12 = mid.tile([128, 2, A, NV], BF16, tag="e12")
                    nc.vector.tensor_scalar(e12[:], stat["su"][:], -1.0, 1.0, op0=AL.mult, op1=AL.add)
                    p_t = mid.tile([128, A, NV], BF16, tag="p")
                    se12 = stat["sе12"]
                    nc.vector.tensor_tensor(p_t[:], se12[:, 0], se12[:, 1], AL.mult)
                    w = mid.tile([128, A, NV], BF16, tag="w")
                    nc.vector.tensor_tensor(w[:], stat["sp"][:], stat["se"][:], AL.mult)
                    for j in range(A):
                        dum = mid.tile([128, NV], BF16, tag="dum")
                        nc.scalar.activation(
                            dum[:], stat["sw"][:, j], AF.Copy,
                            accum_out=res_t[:, c0 + j : c0 + j + 1],
                        )
                nc.sync.dma_start(
                    out=out_d.ap().rearrange("(p c) -> p c", c=C), in_=res_t[:]
                )
                continue

            def stage_a(s):
                c0 = s * A
                if "-big1" in variant:
                    # one whole-pass DMA per tensor: bigger transfers, 1/5th
                    # the fixed DMA costs; chained reps still double-buffer
                    # through the ld pool
                    if s == 0:
                        mb = ld.tile([128, C, MCW], BF16, tag="m")
                        (nc.gpsimd if mc_f8 else nc.sync).dma_start(
                            out=mb[:], in_=mc_v[:, :, 0:MCW]
                        )
                        eb = ld.tile([128, C, NVv], BF16, tag="e")
                        (nc.gpsimd if em_f8 else nc.sync).dma_start(
                            out=eb[:], in_=em_v[:, :, 0:NVv]
                        )
                        stage_state["mb"] = mb
                        stage_state["eb"] = eb
                    mb = stage_state["mb"]
                    eb = stage_state["eb"]
                    e_ap = eb[:, c0 : c0 + A]
                    m0 = mb[:, c0 : c0 + A, 0:NVv]
                    mW = mb[:, c0 : c0 + A, Wv : Wv + NVv]
                    m1 = mb[:, c0 : c0 + A, 1 : 1 + NVv]
                    t12 = mid.tile([128, 2, A, NVv], BF16, tag="t12")
                    nc.vector.tensor_tensor(t12[:, 0], m0, mW, AL.mult)
                    nc.vector.tensor_tensor(t12[:, 1], m0, m1, AL.mult)
                    stage_state[s] = (e_ap, t12, None)
                    return
                m = ld.tile([128, A, MCW], BF16, tag="m")
                # emul: no em tile at all - the em load is a CCE mult into p
                # during stage_c (see below)
                e = None if "-emul" in variant else ld.tile(
                    [128, A, NVv], BF16, tag="e"
                )
                if variant == "tinydma":
                    nc.sync.dma_start(out=m[:, :, 0:16], in_=mc_v[:, c0 : c0 + A, 0:16])
                    nc.sync.dma_start(out=e[:, :, 0:16], in_=em_v[:, c0 : c0 + A, 0:16])
                else:
                    mc_eng = nc.gpsimd if (mc_f8 or mc_q8) else (
                        nc.scalar if "-mcs" in variant else nc.sync
                    )
                    mc_eng.dma_start(out=m[:], in_=mc_v[:, c0 : c0 + A, 0:MCW])
                    if e is not None:
                        (nc.gpsimd if em_f8 else nc.sync).dma_start(
                            out=e[:], in_=em_v[:, c0 : c0 + A, 0:NVv]
                        )
                if variant == "dmaonly" or "-xcomp" in variant:
                    stage_state[s] = (e, None, None)
                    return
                if "-cg" in variant:
                    # t-products fused into the 2X custom e() op in stage_c
                    stage_state[s] = (e, m, None)
                    return
                if "v10" in variant or "v13" in variant:
                    t12 = mid.tile([128, 2, A, NVv], BF16, tag="t12")
                    nc.vector.tensor_tensor(
                        t12[:, 0], m[:, :, 0:NVv], m[:, :, Wv : Wv + NVv], AL.mult
                    )
                    if "-xal" in variant:  # timing probe: aligned in1 (wrong math)
                        nc.vector.tensor_tensor(
                            t12[:, 1], m[:, :, 0:NVv], m[:, :, 0:NVv], AL.mult
                        )
                    elif "-xt2" not in variant:  # timing ablation: drop t2 op
                        nc.vector.tensor_tensor(
                            t12[:, 1], m[:, :, 0:NVv], m[:, :, 1 : 1 + NVv], AL.mult
                        )
                    stage_state[s] = (e, t12, None)
                    return
                t1 = mid.tile([128, A, NV], BF16, tag="t1")
                nc.vector.tensor_tensor(t1[:], m[:, :, 0:NV], m[:, :, W:AREA], AL.mult)
                t2 = mid.tile([128, A, NV], BF16, tag="t2")
                nc.vector.tensor_tensor(t2[:], m[:, :, 0:NV], m[:, :, 1 : 1 + NV], AL.mult)
                stage_state[s] = (e, t1, t2)

            def stage_b(s):
                if variant == "dmaonly" or "-xcomp" in variant:
                    return
                e, t1, t2 = stage_state[s]
                if "-cg" in variant:
                    return
                if "formB" in variant and s == 2:
                    t12 = t1
                    g12 = mid.tile([128, 2, A, NV], BF16, tag="u12")
                    nc.vector.tensor_scalar(
                        g12[:], t12[:], -1.0, 2.0, op0=AL.mult, op1=AL.add
                    )
                    stage_state[s] = (e, t12, g12)
                    return
                if "v10" in variant or "v13" in variant:
                    t12 = t1
                    if "-cp" in variant:
                        if "-eah" in variant:
                            # ACT-offload: for hn of the A area slots, e12 is
                            # computed on ACT (Square then 1-u Copy), freeing
                            # the DVE EE2 op for those slots; DVE later does
                            # just p = e1*e2 (2x tt) for them.
                            hn = int(variant.split("-eah")[1][0])
                            u12 = mid.tile([128, 2, hn, NVv], BF16, tag="u12")
                            nc.scalar.activation(
                                u12[:], t12[:, :, 0:hn], AF.Square,
                                bias=1.0, scale=-1.0,
                            )
                            # e12 = 1-u12 in place (second ACT pass, same tile)
                            nc.scalar.activation(
                                u12[:], u12[:], AF.Copy, bias=1.0, scale=-1.0
                            )
                            stage_state[s] = (e, t12, u12)
                            return
                        # fused custom op computes p straight from t12 in
                        # stage_c; no u12/e12 tiles needed at all
                        stage_state[s] = (e, t12, None)
                        return
                    u12 = mid.tile([128, 2, A, NVv], BF16, tag="u12")
                    e12 = mid.tile([128, 2, A, NVv], BF16, tag="e12")
                    if "flat" in variant:
                        # flat 2D APs so the elementwise map can hit the
                        # fastest DVE perf mode (multi-dim APs cap it)
                        t12f = t12.rearrange("p x a k -> p (x a k)")
                        u12f = u12.rearrange("p x a k -> p (x a k)")
                        e12f = e12.rearrange("p x a k -> p (x a k)")
                        nc.scalar.activation(
                            u12f[:], t12f[:], AF.Square, bias=1.0, scale=-1.0
                        )
                        if s < ea_n:
                            nc.scalar.activation(
                                e12f[:], u12f[:], AF.Copy, bias=1.0, scale=-1.0
                            )
                        else:
                            nc.vector.tensor_scalar(
                                e12f[:], u12f[:], -1.0, 1.0, op0=AL.mult, op1=AL.add
                            )
                        stage_state[s] = (e, e12, None)
                        return
                    nc.scalar.activation(u12[:], t12[:], AF.Square, bias=1.0, scale=-1.0)
                    nc.vector.tensor_scalar(
                        e12[:], u12[:], -1.0, 1.0, op0=AL.mult, op1=AL.add
                    )
                    stage_state[s] = (e, e12, None)
                    return
                u1 = mid.tile([128, A, NV], BF16, tag="u1")
                nc.scalar.activation(u1[:], t1[:], AF.Square, bias=1.0, scale=-1.0)
                u2 = mid.tile([128, A, NV], BF16, tag="u2")
                nc.scalar.activation(u2[:], t2[:], AF.Square, bias=1.0, scale=-1.0)
                if "v9" in variant:
                    # e1,e2 share one tile so p = e1*e2 hits the DVE
                    # single-source fastpath
                    e12 = mid.tile([128, 2, A, NV], BF16, tag="e12")
                    nc.vector.tensor_scalar(
                        e12[:, 0], u1[:], -1.0, 1.0, op0=AL.mult, op1=AL.add
                    )
                    nc.vector.tensor_scalar(
                        e12[:, 1], u2[:], -1.0, 1.0, op0=AL.mult, op1=AL.add
                    )
                    stage_state[s] = (e, e12, None)
                    return
                e1 = mid.tile([128, A, NV], BF16, tag="t1" if reuse else "e1")
                nc.vector.tensor_scalar(e1[:], u1[:], -1.0, 1.0, op0=AL.mult, op1=AL.add)
                e2 = mid.tile([128, A, NV], BF16, tag="t2" if reuse else "e2")
                nc.vector.tensor_scalar(e2[:], u2[:], -1.0, 1.0, op0=AL.mult, op1=AL.add)
                stage_state[s] = (e, e1, e2)

            def stage_c(s):
                c0 = s * A
                e, e1, e2 = stage_state.pop(s)
                if variant == "dmaonly" or "-xcomp" in variant:
                    return
                if "formB" in variant and s == 2:
                    t12, g12 = e1, e2
                    v = mid.tile([128, A, NV], BF16, tag="p")
                    nc.vector.tensor_tensor(v[:], t12[:, 0], t12[:, 1], AL.mult)
                    d = mid.tile([128, A, NV], BF16, tag="d")
                    nc.vector.tensor_tensor(d[:], g12[:, 0], g12[:, 1], AL.mult)
                    p_t = mid.tile([128, A, NV], BF16, tag="p2")
                    nc.vector.tensor_tensor(p_t[:], v[:], d[:], AL.mult)
                    w = mid.tile([128, A, NV], BF16, tag="w")
                    nc.vector.tensor_tensor(w[:], p_t[:], e[:], AL.mult)
                    for j in range(A):
                        dum = mid.tile([128, NV], BF16, tag="dum")
                        nc.scalar.activation(
                            dum[:], w[:, j], AF.Copy,
                            accum_out=res_t[:, c0 + j : c0 + j + 1],
                        )
                    return
                if "v13" in variant:
                    # total-work-lean: per-slot stt fuses w=p*em with the
                    # f32 accumulate; no w tile, no ACT reductions
                    e12 = e1
                    p_t = mid.tile([128, A, NV], BF16, tag="p")
                    nc.vector.tensor_tensor(p_t[:], e12[:, 0], e12[:, 1], AL.mult)
                    for j in range(A):
                        qd = mid.tile([128, NV], BF16, tag="dum")
                        nc.vector.scalar_tensor_tensor(
                            qd[:], p_t[:, j], 1.0, e[:, j],
                            op0=AL.mult, op1=AL.mult,
                            accum_out=res_t[:, c0 + j : c0 + j + 1],
                        )
                    return
                if "v9" in variant or "v10" in variant:
                    e12 = e1
                    p_t = None if "-pip" in variant else mid.tile(
                        [128, A, NVv], BF16, tag="p"
                    )
                    if "-cg" in variant:
                        # g_i = e(t_i) straight from m via the hand-2X custom
                        # op (2 elems/cycle); p = g1*g2 on the stock 2x tt.
                        m = e1
                        if "-psep" in variant:
                            g1_t = mid.tile([128, A, NVv], BF16, tag="t12")
                            g2_t = mid.tile([128, A, NVv], BF16, tag="g2")
                            g_lo, g_hi = g1_t[:], g2_t[:]
                        else:
                            g12 = mid.tile([128, 2, A, NVv], BF16, tag="t12")
                            g_lo, g_hi = g12[:, 0], g12[:, 1]
                        if "-mq8" in variant:
                            gop, gs0, gs1 = ANT_EGQ2X, 2.0, 1.0 / (255.0 * 255.0)
                            kw = {"s1": gs1}
                        else:
                            gop, gs0, kw = ANT_EG2X, 2.0, {}
                        b1 = nc.vector._custom_dve(
                            gop, out=g_lo, in0=m[:, :, 0:NVv],
                            in1=m[:, :, Wv : Wv + NVv], s0=gs0, **kw,
                        )
                        b1.ins.perf_max = 1
                        if "-xg2" not in variant:  # timing ablation
                            b2 = nc.vector._custom_dve(
                                gop, out=g_hi, in0=m[:, :, 0:NVv],
                                in1=m[:, :, 1 : 1 + NVv], s0=gs0, **kw,
                            )
                            b2.ins.perf_max = 1
                        if "-xp" in variant:  # timing ablation: skip p op
                            p_t = g_lo
                        elif "-pip" in variant:
                            # p = g1*g2 in place into g12[:,0]: no p tile at
                            # all, freeing SBUF for deeper ld prefetch (b23)
                            g12v = g12.rearrange("p x a k -> p x (a k)")
                            nc.vector.tensor_tensor(
                                g12v[:, 0], g12v[:, 0], g12v[:, 1], AL.mult
                            )
                            p_t = g12[:, 0]
                        elif "-psep" in variant:
                            nc.vector.tensor_tensor(p_t[:], g1_t[:], g2_t[:], AL.mult)
                        else:
                            g12v = g12.rearrange("p x a k -> p x (a k)")
                            p_f = p_t.rearrange("p a k -> p (a k)")
                            nc.vector.tensor_tensor(
                                p_f[:], g12v[:, 0], g12v[:, 1], AL.mult
                            )
                    elif "-cp" in variant:
                        # p = e(t1)*e(t2) in one fused DVE op (1x rate =
                        # same DVE cycles as the e12 ts + p tt it replaces)
                        # -> ACT's Square over u12 disappears entirely
                        t12v = e12.rearrange("p x a k -> p x (a k)")
                        p_f = p_t.rearrange("p a k -> p (a k)")
                        if e2 is not None:  # -eah: first hn slots via ACT e12
                            hn = e2.shape[2]
                            nc.vector.tensor_tensor(
                                p_t[:, 0:hn], e2[:, 0], e2[:, 1], AL.mult
                            )
                            nc.vector._custom_dve(
                                ANT_EE2,
                                out=p_t[:, hn:A],
                                in0=e12[:, 0, hn:A],
                                in1=e12[:, 1, hn:A],
                            )
                        elif "-xee2" in variant:  # timing ablation: stock tt
                            nc.vector.tensor_tensor(
                                p_f[:], t12v[:, 0], t12v[:, 1], AL.mult
                            )
                        else:
                            nc.vector._custom_dve(
                                ANT_EE2, out=p_f[:], in0=t12v[:, 0], in1=t12v[:, 1]
                            )
                        if "-caw" in variant:
                            # w = p*em fused with the per-area reduction in
                            # one DVE op each: ACT leaves the steady state
                            for j in range(A):
                                dum = mid.tile([128, NVv], BF16, tag="dum")
                                nc.vector.affine_mul_reduce(
                                    out=dum[:],
                                    accum_out=res_t[:, c0 + j : c0 + j + 1],
                                    in0=p_t[:, j],
                                    in1=e[:, j],
                                    scale=1.0,
                                    bias=0.0,
                                )
                            return
                    else:
                        nc.vector.tensor_tensor(p_t[:], e12[:, 0], e12[:, 1], AL.mult)
                    if "-emul" in variant:
                        # w = p*em computed by the SWDGE CCE during the em
                        # load itself: DRAM em (fp8, cast) multiplies into
                        # the p tile. No em SBUF tile, no DVE w op.
                        nc.gpsimd.dma_start(
                            out=p_t[:], in_=em_v[:, c0 : c0 + A, 0:NVv],
                            accum_op=AL.mult,
                        )
                        w = p_t
                    elif "-w7" in variant:
                        # w computed by the DMA compute engine (CCE mult),
                        # in-place into the em tile: frees ~4800c of DVE
                        nc.gpsimd.dma_start(out=e[:], in_=p_t[:], accum_op=AL.mult)
                        w = e
                    elif "-xw" in variant:  # timing ablation: drop w op
                        w = p_t
                    elif "-wip" in variant:
                        # w = p*em in place into the p tile (one less tile)
                        nc.vector.tensor_tensor(p_t[:], p_t[:], e[:], AL.mult)
                        w = p_t
                    else:
                        w = mid.tile([128, A, NVv], BF16, tag="w")
                        nc.vector.tensor_tensor(w[:], p_t[:], e[:], AL.mult)
                    if "-xred" in variant:  # timing ablation: no reductions
                        return
                    if "fold1" in variant:
                        # halve reduction length with one same-tile tt-add
                        wh = mid.tile([128, A, NV // 2], BF16, tag="wh")
                        nc.vector.tensor_tensor(
                            wh[:], w[:, :, 0 : NV // 2], w[:, :, NV // 2 : NV], AL.add
                        )
                        for j in range(A):
                            dum = mid.tile([128, NV // 2], BF16, tag="dumh")
                            nc.scalar.activation(
                                dum[:], wh[:, j], AF.Copy,
                                accum_out=res_t[:, c0 + j : c0 + j + 1],
                            )
                        return
                    for j in range(A):
                        if "k0" not in variant and s in (1, 3) and j == 0:
                            qd = mid.tile([128, NVv], BF16, tag="dum")
                            nc.vector.tensor_scalar(
                                qd[:], w[:, j], 1.0, None, op0=AL.mult, op1=AL.add,
                                accum_out=res_t[:, c0 + j : c0 + j + 1],
                            )
                        elif "pdum" in variant:
                            # acc dummy write goes to PSUM: off the SBUF ports
                            dum = psp.tile([128, NVv], F32, tag="pdum")
                            nc.scalar.activation(
                                dum[:], w[:, j], AF.Copy,
                                accum_out=res_t[:, c0 + j : c0 + j + 1],
                            )
                        elif "f8dum" in variant:
                            dum = mid.tile([128, NVv], mybir.dt.float8e4, tag="dum8f")
                            nc.scalar.activation(
                                dum[:], w[:, j], AF.Copy,
                                accum_out=res_t[:, c0 + j : c0 + j + 1],
                            )
                        else:
                            dum = mid.tile([128, NVv], BF16, tag="dum")
                            nc.scalar.activation(
                                dum[:], w[:, j], AF.Copy,
                                accum_out=res_t[:, c0 + j : c0 + j + 1],
                            )
                    return
                p_t = mid.tile([128, A, NV], BF16, tag="u1" if reuse else "p")
                nc.vector.tensor_tensor(p_t[:], e1[:], e2[:], AL.mult)
                if "v8" in variant:
                    # w = p*em on DVE, then DMA-CCE tree-fold (add) 992->124
                    # so the per-area reductions touch only 124 elements
                    w = mid.tile([128, A, NV], BF16, tag="u2" if reuse else "w")
                    nc.vector.tensor_tensor(w[:], p_t[:], e[:], AL.mult)
                    nc.gpsimd.dma_start(
                        out=w[:, :, 0:496], in_=w[:, :, 496:992], accum_op=AL.add
                    )
                    nc.gpsimd.dma_start(
                        out=w[:, :, 0:248], in_=w[:, :, 248:496], accum_op=AL.add
                    )
                    nc.gpsimd.dma_start(
                        out=w[:, :, 0:124], in_=w[:, :, 124:248], accum_op=AL.add
                    )
                    for j in range(A):
                        dum = mid.tile([128, 124], BF16, tag="dum8")
                        nc.scalar.activation(
                            dum[:], w[:, j, 0:124], AF.Copy,
                            accum_out=res_t[:, c0 + j : c0 + j + 1],
                        )
                    return
                if "v7" in variant:
                    # w = p*em computed by the DMA compute engine, in-place
                    # into the em tile; reductions split DVE/ACT per slot
                    nc.gpsimd.dma_start(out=e[:], in_=p_t[:], accum_op=AL.mult)
                    for j in range(A):
                        if j % 2 == 0:
                            qd = mid.tile([128, NV], BF16, tag="dum")
                            nc.vector.tensor_scalar(
                                qd[:], e[:, j], 1.0, None, op0=AL.mult,
                                op1=AL.add,
                                accum_out=res_t[:, c0 + j : c0 + j + 1],
                            )
                        else:
                            dum = mid.tile([128, NV], BF16, tag="dum")
                            nc.scalar.activation(
                                dum[:], e[:, j], AF.Copy,
                                accum_out=res_t[:, c0 + j : c0 + j + 1],
                            )
                    return
                w = mid.tile([128, A, NV], BF16, tag="u2" if reuse else "w")
                if "gw" in variant:
                    nc.gpsimd.tensor_tensor(w[:], p_t[:], e[:], AL.mult)
                else:
                    nc.vector.tensor_tensor(w[:], p_t[:], e[:], AL.mult)
                for j in range(A):
                    dum = mid.tile([128, NV], BF16, tag="dum")
                    nc.scalar.activation(
                        dum[:], w[:, j], AF.Copy,
                        accum_out=res_t[:, c0 + j : c0 + j + 1],
                    )

            for s in range(n_super + lag_c):
                if s < n_super:
                    stage_a(s)
                if 1 <= s < n_super + 1:
                    stage_b(s - 1)
                if s >= lag_c:
                    stage_c(s - lag_c)

            if variant == "dmaonly" or "-xred" in variant or "-xcomp" in variant:
                nc.gpsimd.memset(res_t[:], 0.0)
            if variant != "lastout" or _rr == inner_reps - 1:
                nc.sync.dma_start(
                    out=out_d.ap().rearrange("(p c) -> p c", c=C), in_=res_t[:]
                )

    nc.compile()
    return nc

class _Exec:
    """One-time-jitted SPMD executor for a prebuilt Bass graph.

    Vendored from bass2jax.run_bass_via_pjrt so repeated calls reuse the
    compiled executable (run_bass_via_pjrt re-jits per invocation)."""

    def __init__(self, nc: bass.Bass, n_cores: int):
        bass2jax.install_neuronx_cc_hook()
        assert nc.dbg_addr is None or not nc.dbg_callbacks
        partition_name = (
            nc.partition_id_tensor.name if nc.partition_id_tensor else None
        )
        in_names, out_names, out_avals = [], [], []
        for alloc in nc.m.functions[0].allocations:
            if not isinstance(alloc, mybir.MemoryLocationSet):
                continue
            name = alloc.memorylocations[0].name
            if alloc.kind == "ExternalInput":
                if name != partition_name and name != getattr(nc.dbg_addr, "name", None):
                    in_names.append(name)
            elif alloc.kind == "ExternalOutput":
                shape = tuple(alloc.tensor_shape)
                dtype = mybir.dt.np(alloc.dtype)
                out_names.append(name)
                out_avals.append(jax.core.ShapedArray(shape, dtype))
        self.in_names = list(in_names)
        self.out_names = out_names
        self.out_avals = out_avals
        self.n_cores = n_cores
        n_params = len(in_names)
        n_outs = len(out_avals)

        all_in_names = list(in_names) + list(out_names)
        if nc.dbg_addr is not None:
            all_in_names.append(nc.dbg_addr.name)
        if partition_name is not None:
            all_in_names.append(partition_name)
        self._has_dbg = nc.dbg_addr is not None

        def jnp_zeros_dbg():
            import jax.numpy as jnp

            return jnp.zeros((1, 2), np.uint32)

        def _call_once(ins, outs):
            operands = list(ins) + list(outs)
            if self._has_dbg:
                operands.append(jnp_zeros_dbg())
            if partition_name is not None:
                operands.append(bass2jax.partition_id_tensor())
            return tuple(
                bass2jax._bass_exec_p.bind(
                    *operands,
                    out_avals=tuple(out_avals),
                    in_names=tuple(all_in_names),
                    out_names=tuple(out_names),
                    lowering_input_output_aliases=(),
                    sim_require_finite=True,
                    sim_require_nnan=True,
                    nc=nc,
                )
            )

        self._call_once = _call_once

        def _body(*args):
            return _call_once(args[:n_params], args[n_params:])

        devices = jax.devices()[:n_cores]
        assert len(devices) == n_cores
        self.mesh = Mesh(np.asarray(devices), ("core",))
        in_specs = (PartitionSpec("core"),) * (n_params + n_outs)
        out_specs = (PartitionSpec("core"),) * n_outs
        donate = tuple(range(n_params, n_params + n_outs))
        self._fn = jax.jit(
            shard_map(
                _body,
                mesh=self.mesh,
                in_specs=in_specs,
                out_specs=out_specs,
                check_rep=False,
            ),
            donate_argnums=donate,
            keep_unused=True,
        )
        self.sharding = NamedSharding(self.mesh, PartitionSpec("core"))
        self._n_params = n_params
        self._n_outs = n_outs
        self._in_specs = in_specs
        self._chain_cache = {}

    def chain_fn(self, n: int):
        """Jitted fn executing the NEFF n times, serialized via the out bufs."""
        if n not in self._chain_cache:
            def _chain_body(*args):
                ins = args[: self._n_params]
                outs = tuple(args[self._n_params :])
                for _ in range(n):
                    outs = self._call_once(ins, outs)
                return outs

            donate = tuple(range(self._n_params, self._n_params + self._n_outs))
            self._chain_cache[n] = jax.jit(
                shard_map(
                    _chain_body,
                    mesh=self.mesh,
                    in_specs=self._in_specs,
                    out_specs=(PartitionSpec("core"),) * self._n_outs,
                    check_rep=False,
                ),
                donate_argnums=donate,
                keep_unused=True,
            )
        return self._chain_cache[n]

    def time_chain(self, concat_in_dev, n: int, reps: int = 10):
        import time

        fn = self.chain_fn(n)
        for _ in range(2):
            jax.block_until_ready(fn(*concat_in_dev, *self.fresh_zeros()))
        times = []
        for _ in range(reps):
            zeros = self.fresh_zeros()
            jax.block_until_ready(zeros)
            t0 = time.perf_counter()
            jax.block_until_ready(fn(*concat_in_dev, *zeros))
            times.append(time.perf_counter() - t0)
        return min(times)

    def concat_inputs(self, in_maps):
        return [
            np.concatenate([np.asarray(m[name]) for m in in_maps], axis=0)
            for name in self.in_names
        ]

    def fresh_zeros(self):
        return [
            jax.device_put(
                np.zeros((self.n_cores * a.shape[0], *a.shape[1:]), a.dtype),
                self.sharding,
            )
            for a in self.out_avals
        ]

    def __call__(self, concat_in):
        out_arrs = self._fn(*concat_in, *self.fresh_zeros())
        return [np.asarray(o) for o in out_arrs]


_EXEC_CACHE = {}


def _get_exec(shard: int, inner_reps: int = 1, variant: str | None = None) -> _Exec:
    if variant is None:
        variant = DEFAULT_VARIANT
    key = (shard, inner_reps, variant)
    if key not in _EXEC_CACHE:
        _EXEC_CACHE[key] = _Exec(_build(shard, inner_reps, variant=variant), N_CORES)
    return _EXEC_CACHE[key]


def _benchmark(mask_combined, edge_map, mask_index=1, inner_reps=257, reps=40, variant=None):
    """Measure steady-state per-pass device time by comparing a 1-rep NEFF
    against an inner_reps-rep NEFF. Calls are interleaved pairwise and the
    median difference is used, cancelling the multi-ms drift of the ~80 ms
    axon dispatch overhead."""
    import time

    full, shard = _prep_inputs(mask_combined, edge_map, mask_index, variant)
    ex1 = _get_exec(SHARD_PAD, 1, variant=variant)
    exR = _get_exec(SHARD_PAD, inner_reps, variant=variant)
    dev_in = [jax.device_put(full[name], ex1.sharding) for name in ex1.in_names]
    fn1 = ex1.chain_fn(1)
    fnR = exR.chain_fn(1)

    def one(ex, fn):
        z = ex.fresh_zeros()
        jax.block_until_ready(z)
        t0 = time.perf_counter()
        jax.block_until_ready(fn(*dev_in, *z))
        return time.perf_counter() - t0

    for _ in range(3):
        one(ex1, fn1)
        one(exR, fnR)
    diffs = []
    t1s = []
    for _ in range(reps):
        a = one(ex1, fn1)
        b = one(exR, fnR)
        t1s.append(a)
        diffs.append(b - a)
    diffs = np.array(diffs)
    loop_ns = float(np.median(diffs)) / (inner_reps - 1) * 1e9
    return {
        "dispatch_1rep_ns": float(np.min(t1s)) * 1e9,
        "diff_med_ns": float(np.median(diffs)) * 1e9,
        "diff_p25_ns": float(np.percentile(diffs, 25)) * 1e9,
        "diff_p75_ns": float(np.percentile(diffs, 75)) * 1e9,
        "loop_ns": loop_ns,
    }


def _prep_inputs(mask_combined, edge_map, mask_index, variant=None):
    import ml_dtypes

    if variant is None:
        variant = DEFAULT_VARIANT
    bf16 = ml_dtypes.bfloat16
    idx = int(np.asarray(mask_index))
    # convert up front: jax-array inputs would otherwise dispatch jax ops
    # (potentially on the neuron backend) during host-side prep
    mask_combined = np.asarray(mask_combined)
    edge_map = np.asarray(edge_map)
    B = mask_combined.shape[0]
    assert B % N_CORES == 0, B
    shard = B // N_CORES
    assert shard <= SHARD_PAD
    mc = np.asarray(mask_combined[..., idx], dtype=np.float32).astype(bf16)
    mc[:, :, -1] = 0  # reference zeroes last col/row of the selected mask
    mc[:, -1, :] = 0
    if "p31" in variant:
        # dense 31-stride packing: mc rows 0..30 cols 0..30 (961), em rows
        # 0..29 at 31-wide with col 30 zeroed (930). Same math, 3% fewer
        # elements in every stream.
        mc = np.ascontiguousarray(mc[:, 0:31, 0:31]).reshape(B, 961)
        em2 = np.asarray(edge_map, dtype=np.float32)[..., 0]
        em = np.zeros((B, 30, 31), bf16)
        em[:, :, 0:30] = em2[:, 0:30, 0:30].astype(bf16)
        em = em.reshape(B, 930)
    else:
        mc = mc.reshape(B, AREA)
        em = (
            np.asarray(edge_map, dtype=np.float32)[..., 0]
            .reshape(B, AREA)[:, :NV]
            .astype(bf16)
        )
    if "-mq8" in variant:
        # uint8 fixed-point: m ~ q/255, round to nearest. RMS quantization
        # error ~1.1e-3 absolute, ~20x better than fp8e4m3 at 1 byte/elem.
        mc = np.rint(mc.astype(np.float32) * 255.0).clip(0, 255).astype(np.uint8)
    elif "-f8" in variant:
        mc = mc.astype(ml_dtypes.float8_e4m3)
    if "-f8" in variant or "-ef8" in variant:
        em = em.astype(ml_dtypes.float8_e4m3)

    # pad each core's shard to SHARD_PAD rows of zeros (zero areas -> zero loss)
    def pad(x):
        x = x.reshape(N_CORES, shard, x.shape[-1])
        out = np.zeros((N_CORES, SHARD_PAD, x.shape[-1]), x.dtype)
        out[:, :shard] = x
        return out.reshape(N_CORES * SHARD_PAD, x.shape[-1])

    return {"mc": pad(mc), "em": pad(em)}, shard


def _run(resized_image=None, mask_combined=None, edge_map=None, mask_index=1, variant=None, **_):
    full, shard = _prep_inputs(mask_combined, edge_map, mask_index, variant)
    ex = _get_exec(SHARD_PAD, variant=variant)
    concat_in = [full[name] for name in ex.in_names]
    outs = ex(concat_in)
    out = outs[ex.out_names.index("out")].reshape(N_CORES, SHARD_PAD)[:, :shard]
    return out.reshape(-1).astype(np.float32, copy=False), ex


# fallback chain: previous proven custom-op config (ANT_EE2, no hand-2X),
# then a stock-op-only variant (no custom DVE, no fp8) as final insurance
FALLBACK_VARIANTS = [
    "v10-k0-b33-flat-p31-cg-a5-ef8",
    "v10-k0-b33-flat-p31-cp-a5-ef8",
    "v10-k0-b53-flat-p31",
]
FALLBACK_VARIANT = FALLBACK_VARIANTS[-1]


def kernel(**inputs) -> np.ndarray:
    try:
        out, _ = _run(**inputs)
        return out
    except Exception:
        pass
    for v in FALLBACK_VARIANTS:
        try:
            out, _ = _run(variant=v, **inputs)
            return out
        except Exception:
            if v == FALLBACK_VARIANTS[-1]:
                raise
    raise RuntimeError("unreachable")


def _time_reps(resized_image=None, mask_combined=None, edge_map=None, mask_index=1, reps=30, **_):
    import time

    full, shard = _prep_inputs(mask_combined, edge_map, mask_index)
    ex = _get_exec(shard)
    concat_in = [
        jax.device_put(full[name], ex.sharding) for name in ex.in_names
    ]
    for _i in range(3):
        jax.block_until_ready(ex._fn(*concat_in, *ex.fresh_zeros()))
    times = []
    for _i in range(reps):
        zeros = ex.fresh_zeros()
        jax.block_until_ready(zeros)
        t0 = time.perf_counter()
        jax.block_until_ready(ex._fn(*concat_in, *zeros))
        times.append(time.perf_counter() - t0)
    return times


def _build_null() -> bass.Bass:
    nc = bacc.Bacc("TRN2", target_bir_lowering=False, debug=False)
    x_d = nc.declare_dram_parameter("x", [128, 8], F32, isOutput=False)
    y_d = nc.declare_dram_parameter("y", [128, 8], F32, isOutput=True)
    with tile.TileContext(nc) as tc:
        with tc.tile_pool(name="p", bufs=1) as pool:
            t = pool.tile([128, 8], F32)
            nc.sync.dma_start(out=t[:], in_=x_d.ap()[:])
            nc.sync.dma_start(out=y_d.ap()[:], in_=t[:])
    nc.compile()
    return nc


def _time_null(reps=30):
    import time

    if "null" not in _EXEC_CACHE:
        _EXEC_CACHE["null"] = _Exec(_build_null(), N_CORES)
    ex = _EXEC_CACHE["null"]
    x = np.zeros((N_CORES * 128, 8), np.float32)
    concat_in = [jax.device_put(x, ex.sharding)]
    for _i in range(3):
        jax.block_until_ready(ex._fn(*concat_in, *ex.fresh_zeros()))
    times = []
    for _i in range(reps):
        zeros = ex.fresh_zeros()
        jax.block_until_ready(zeros)
        t0 = time.perf_counter()
        jax.block_until_ready(ex._fn(*concat_in, *zeros))
        times.append(time.perf_counter() - t0)
    return times



# revision 31
# speedup vs baseline: 1.5076x; 1.5076x over previous
"""Trainium2 Bass kernel for nn_Apply_on_single_area.

Computes, per supervoxel area b:
    loss[b] = sum_{i,j} eroded(mc)[i,j] * em[i,j]
where mc = mask_combined[..., mask_index] with last row/col zeroed and
eroded = E(a1)*E(a2), E(a) = 2a - a^2, a1/a2 = products with the next
element along each spatial axis (zero-padded).

Key simplifications / design (HW-measured on TRN2):
- differentiable_or_simple(a,b) = a*b + (1-a)*a + (1-b)*a = 2a - a^2:
  the b-terms cancel, so only forward-neighbor products a1, a2 matter.
- Only rows/cols 0..29 can contribute (row/col 31 are zeroed, which
  forces e=0 on row 30 / col 30 as well), so the host packs mc at
  31-wide stride (31x31=961) and em as 30x31=930 with col 30 zeroed
  ("p31"): flat shifts become +31/+1 and every stream shrinks ~6%.
- Pure data parallel: B=10000 split 1250/core over 8 cores, padded to
  1280 = 128 partitions x 10 areas, partition-major so every DMA is
  contiguous per partition.
- bf16 compute, f32 accumulation; em is stored fp8e4m3 in DRAM and
  cast to bf16 *by the DMA engine* during the load (SWDGE dtype-cast,
  "ef8"): -24% HBM bytes at zero compute cost, rel err 4.6e-3 vs the
  2e-2 gate (exact-sim'd on the real data before adoption).
- DVE is the bottleneck engine; the "cg" design cuts its cycles 20%+
  below the previous ANT_EE2 config by fusing the shift-product INTO
  the e() evaluation with a runtime-registered custom DVE op ANT_EG2X:
  g = (Src0*Src1)*(2 - Src0*Src1) = e(t), 3 ALU stages, with a
  HAND-AUTHORED 2X_1PORT uop (2 elems/cycle; the body replayed at
  stages 3-5 for the odd element, even result parked in delay lane 0,
  out LO<-DELAY_0 / HI<-ALU_OUT - mirrors the stock 3-ALU op at
  table_ptr 104; perf_max=1 set on the instruction enables the mode).
  Verified bit-level on HW: rel err identical to the stock path.
  Per area the DVE does g1 = EG(m0,m31), g2 = EG(m0,m1) (2X custom),
  p = g1*g2 and w = p*em (stock 2x tt); ACT does the per-area
  Copy+accum reductions (measured off the critical path).
- One A=10 supertile per pass ("a10", w multiplied in place into the p
  tile "wip" so mid/ld pools at 2/2 "b22" fit SBUF): halves the DVE
  instruction-dispatch overhead vs a5; chained reps still pipeline
  through the pools. GPSIMD is only the SWDGE cast-DMA queue; gpsimd
  *compute* measured ~10us slower (Multiply impl efficiency 0.42).
- HW-measured ablations (noisy axon timing, pairwise-interleaved
  medians): ANT_EE2 1x pass ~5us marginal, each t12 tt ~3-6us, ACT
  reduces ~0 (slack), DMA-only floor ~10.4us at the 3.65MB/core/pass
  stream. cg measured 18.1us vs cp 24.2us in the same window (-25%),
  and cg-a10-wip beat cg-a5 in two independent windows (-0.9/-1.7us);
  the harness-scale equivalent of cp was 14.5us. Tested and rejected
  at this operating point (all within noise of the default, so the
  kernel sits at the DVE roofline for its op structure): mc-fp8 "f8"
  (1.18e-2, thin gate margin), mc-uint8 fixed-point "mq8" via
  ANT_EGQ2X (6.1e-3, works but no speed win - loop is not DMA-bound),
  separate g1/g2 tiles "psep", ACT e12 offload "eahN" (ACT saturates),
  DMA-CCE em-multiply (compiler rejects cast+mult), gpsimd compute,
  ld-pool depth 3 "b23" (deeper DMA prefetch: neutral, DMA is not
  the constraint), PSUM/fp8 dummy accum tiles "pdum"/"f8dum" (no SBUF
  write-port contention). Adopted: in-place p into g12 "pip" (drops
  the p tile; <= the non-pip config in every window, -0.3/-0.5us).
  Stock instructions have no perf_max field, so the stock tt ops
  cannot be forced into the 4X_2P table slots. 4 DVE passes/area is
  minimal for 2-source ops: a dual-output g1+g2 op needs 12 > 8 ALU
  stages, and any 3-op tree loses m0 before the second erosion term.
Fallback chain: cg-a5, then cp (ANT_EE2, harness-proven), then a
stock-op-only variant.
"""

import numpy as np

import jax
from jax.experimental.shard_map import shard_map
from jax.sharding import Mesh, NamedSharding, PartitionSpec

import concourse.bass as bass
import concourse.bacc as bacc
import concourse.mybir as mybir
import concourse.tile as tile
from concourse import bass2jax


def _register_ee2():
    """Custom DVE op: out = e(Src0)*e(Src1), e(t) = 1-(1-t)^2.

    Fuses the ACT Square (u12), DVE tensor_scalar (e12) and DVE
    tensor_tensor (p) into one 1x-rate DVE pass: same DVE cycles as the
    e12+p pair it replaces, but removes u12 (2/3 of ACT work) entirely.
    Registered at import so the op's table rows ship in our NEFF; sha is
    pinned from a fresh lower() (semantics verified against reference)."""
    from concourse import dve_ops
    from concourse.dve_spec import Spec, Src0, Src1, One, lower
    from concourse.dve_uop import DveOpSpec

    if any(op.name == "ANT_EE2" for op in dve_ops.OPS):
        return next(op for op in dve_ops.OPS if op.name == "ANT_EE2")

    a1 = One - Src0
    u1 = a1 * a1
    a2 = One - Src1
    u2 = a2 * a2
    spec = Spec(
        body=(One - u1) * (One - u2),
        reference=lambda in0, in1: (1 - (1 - in0) ** 2) * (1 - (1 - in1) ** 2),
    )
    tmp = dve_ops.DveOp("ANT_EE2", spec, subdim=False, uops_sha={})
    dve_ops.OPS.append(tmp)
    dve_ops._SUB_OPCODE_FOR_NAME["ANT_EE2"] = (
        dve_ops._CUSTOM_DVE_ROW_BASE + len(dve_ops.OPS) - 1
    )
    opcode = dve_ops.get_dve_sub_opcode("ANT_EE2")
    shas = {}
    for ver in ("v3", "v4"):
        ds = DveOpSpec(
            name="ANT_EE2", opcode=opcode, uops=lower(spec, ver=ver), rd1_en=True
        )
        shas[ver] = ds.sha(ver)
    final = dve_ops.DveOp("ANT_EE2", spec, subdim=False, uops_sha=shas)
    dve_ops.OPS[-1] = final
    return final


ANT_EE2 = _register_ee2()


def _register_eg2x():
    """Custom DVE op ANT_EG2X: out = g(Src0*Src1), g(t) = t*(s0-t), WITH a
    hand-authored 2X_1PORT uop (2 elems/cycle for bf16 packed operands).

    g(t) with s0=2 equals e(t) = 1-(1-t)^2, so e(t1) = EG(m0, mW) fuses the
    t-product INTO the e() evaluation. Body is 3 ALUs (mult, sub, mult) ->
    the 2X variant replays it at stages 3-5 for the odd element (inputs via
    SRC_0_HI/SRC_1_HI delay lanes, even result parked in d0, out LO<-DELAY_0
    HI<-ALU_OUT), mirroring the stock 3-ALU op at table_ptr 104. perf_max=1
    on the instruction caps the engine at 2X_1P (2X_2P/4X slots hold the
    same uop as don't-care fallbacks but are unreachable)."""
    from concourse import dve_ops
    from concourse.dve_spec import Spec, Src0, Src1, C0, lower
    from concourse.dve_uop import (
        AluInp,
        AluOp,
        DveOpSpec,
        InpSel,
        OutPath,
        OutSel,
        Trigger,
        UopConfig,
        UopDpConfig,
    )

    NAME = "ANT_EG2X"
    if any(op.name == NAME for op in dve_ops.OPS):
        return next(op for op in dve_ops.OPS if op.name == NAME)

    t = Src0 * Src1
    spec = Spec(
        body=t * (C0 - t),
        reference=lambda in0, in1, s0: (in0 * in1) * (s0 - in0 * in1),
    )

    def PD(i):
        return AluInp(AluInp.PREV_DELAY_0 + i)

    PASS = 5  # DelayInp.PREV_DELAY
    CAP = 0  # DelayInp.PREV_ALU_OUT

    def blk(op=AluOp.BYPASS, s0=AluInp.PREV_ALU_OUT, s1=AluInp.PREV_ALU_OUT,
            d=(), cap=()):
        delay = [5] * 7
        delay_enable = [0] * 7
        for i in d:
            delay[i] = PASS
            delay_enable[i] = 1
        for i in cap:
            delay[i] = CAP
            delay_enable[i] = 1
        from concourse.dve_uop import DelayInp
        return UopDpConfig(
            op=op, alu_src0=s0, alu_src1=s1,
            delay=[DelayInp(x) for x in delay],
            alu_out_enable=1, delay_enable=delay_enable,
        )

    M, S = AluOp.MULTIPLY, AluOp.SUBTRACT
    uop2x = UopConfig(
        inp=[InpSel.ZERO, InpSel.SRC_0, InpSel.SRC_1, InpSel.CONST_0,
             InpSel.SRC_0_HI, InpSel.SRC_1_HI, InpSel.ZERO, InpSel.ZERO],
        inp_enable=[0, 1, 1, 1, 1, 1, 0, 0],
        out={OutPath.WR0_LO: OutSel.DELAY_0, OutPath.WR0_HI: OutSel.ALU_OUT,
             OutPath.WR1_LO: OutSel.ALU_OUT, OutPath.WR1_HI: OutSel.ALU_OUT},
        out_enable={OutPath.WR0_LO: 1, OutPath.WR0_HI: 1,
                    OutPath.WR1_LO: 0, OutPath.WR1_HI: 0},
        require_inp0=1, require_inp1=1,
        trigger=(Trigger.SRC_TENSOR_DONE, Trigger.NONE, Trigger.NONE),
        next_uop=(0, 0, 0), repeat_count=0,
        datapath_config=[
            # stages 0-2: even element (same as REGULAR), B inputs ride d3,d4
            blk(M, PD(0), PD(1), d=(1, 2, 3, 4)),          # t_A = s0*s1
            blk(S, PD(2), AluInp.PREV_ALU_OUT, d=(2, 3, 4), cap=(0,)),  # C0-t_A; d0<-t_A
            blk(M, PD(0), AluInp.PREV_ALU_OUT, d=(2, 3, 4)),  # g_A = t_A*(C0-t_A)
            # stages 3-5: odd element; g_A parked in d0
            blk(M, PD(3), PD(4), d=(2,), cap=(0,)),          # t_B; d0<-g_A
            blk(S, PD(2), AluInp.PREV_ALU_OUT, d=(0,), cap=(1,)),  # C0-t_B; d1<-t_B
            blk(M, PD(1), AluInp.PREV_ALU_OUT, d=(0,)),       # g_B
            blk(d=(0,)),                                        # pass g_B + d0
            blk(d=(0,)),
        ],
    )
    uop2x.validate("v3")

    class DveOp2x:
        name = NAME
        subdim = False

        def __init__(self):
            self.spec = spec
            self._cache = {}

        def compile(self, ver):
            if ver in self._cache:
                return self._cache[ver]
            s = DveOpSpec(
                name=NAME,
                opcode=dve_ops.get_dve_sub_opcode(NAME),
                uops=lower(spec, ver=ver),
                rd1_en=True,
                uops_2x=[uop2x] if ver == "v3" else None,
            )
            self._cache[ver] = s
            return s

        def validate(self, ver):
            return self.compile(ver).validate(ver)

    dve_ops._SUB_OPCODE_FOR_NAME[NAME] = (
        dve_ops._CUSTOM_DVE_ROW_BASE + len(dve_ops.OPS)
    )
    op = DveOp2x()
    dve_ops.OPS.append(op)
    return op


ANT_EG2X = _register_eg2x()


def _register_egq2x():
    """ANT_EGQ2X: out = g(Src0*Src1*s1), g(u) = u*(s0-u), 4 ALUs, with a
    hand-authored 2X_1PORT uop (8 stages exactly). For uint8 fixed-point mc:
    m = q/255 -> s1 = 1/255^2, s0 = 2 gives g = e(m0*m1) with ~20x lower
    RMS quantization error than fp8e4m3 at the same 1 byte/elem."""
    from concourse import dve_ops
    from concourse.dve_spec import Spec, Src0, Src1, C0, C1, lower
    from concourse.dve_uop import (
        AluInp, AluOp, DelayInp, DveOpSpec, InpSel, OutPath, OutSel,
        Trigger, UopConfig, UopDpConfig,
    )

    NAME = "ANT_EGQ2X"
    if any(op.name == NAME for op in dve_ops.OPS):
        return next(op for op in dve_ops.OPS if op.name == NAME)

    t = Src0 * Src1
    u = t * C1
    spec = Spec(
        body=u * (C0 - u),
        reference=lambda in0, in1, s0, s1: (in0 * in1 * s1)
        * (s0 - in0 * in1 * s1),
    )

    def PD(i):
        return AluInp(AluInp.PREV_DELAY_0 + i)

    def blk(op=AluOp.BYPASS, s0=AluInp.PREV_ALU_OUT, s1=AluInp.PREV_ALU_OUT,
            d=(), cap=()):
        delay = [DelayInp.PREV_DELAY] * 7
        delay_enable = [0] * 7
        for i in d:
            delay_enable[i] = 1
        for i in cap:
            delay[i] = DelayInp.PREV_ALU_OUT
            delay_enable[i] = 1
        return UopDpConfig(op=op, alu_src0=s0, alu_src1=s1, delay=delay,
                           alu_out_enable=1, delay_enable=delay_enable)

    M, S = AluOp.MULTIPLY, AluOp.SUBTRACT
    # lanes: d0=s0 d1=s1 d2=C1 d3=C0 d4=s0_HI d5=s1_HI (matches lower()'s
    # REGULAR lane plan extended with the odd element)
    uop2x = UopConfig(
        inp=[InpSel.ZERO, InpSel.SRC_0, InpSel.SRC_1, InpSel.CONST_1,
             InpSel.CONST_0, InpSel.SRC_0_HI, InpSel.SRC_1_HI, InpSel.ZERO],
        inp_enable=[0, 1, 1, 1, 1, 1, 1, 0],
        out={OutPath.WR0_LO: OutSel.DELAY_0, OutPath.WR0_HI: OutSel.ALU_OUT,
             OutPath.WR1_LO: OutSel.ALU_OUT, OutPath.WR1_HI: OutSel.ALU_OUT},
        out_enable={OutPath.WR0_LO: 1, OutPath.WR0_HI: 1,
                    OutPath.WR1_LO: 0, OutPath.WR1_HI: 0},
        require_inp0=1, require_inp1=1,
        trigger=(Trigger.SRC_TENSOR_DONE, Trigger.NONE, Trigger.NONE),
        next_uop=(0, 0, 0), repeat_count=0,
        datapath_config=[
            blk(M, PD(0), PD(1), d=(2, 3, 4, 5)),           # t_A
            blk(M, AluInp.PREV_ALU_OUT, PD(2), d=(2, 3, 4, 5)),  # u_A = t_A*C1
            blk(S, PD(3), AluInp.PREV_ALU_OUT, d=(2, 3, 4, 5), cap=(0,)),  # C0-u_A; d0<-u_A
            blk(M, PD(0), AluInp.PREV_ALU_OUT, d=(2, 3, 4, 5)),  # g_A
            blk(M, PD(4), PD(5), d=(2, 3), cap=(0,)),       # t_B; d0<-g_A
            blk(M, AluInp.PREV_ALU_OUT, PD(2), d=(0, 3)),   # u_B
            blk(S, PD(3), AluInp.PREV_ALU_OUT, d=(0,), cap=(1,)),  # C0-u_B; d1<-u_B
            blk(M, PD(1), AluInp.PREV_ALU_OUT, d=(0,)),     # g_B
        ],
    )
    uop2x.validate("v3")

    class DveOpQ2x:
        name = NAME
        subdim = False

        def __init__(self):
            self.spec = spec
            self._cache = {}

        def compile(self, ver):
            if ver in self._cache:
                return self._cache[ver]
            s = DveOpSpec(
                name=NAME,
                opcode=dve_ops.get_dve_sub_opcode(NAME),
                uops=lower(spec, ver=ver),
                rd1_en=True,
                uops_2x=[uop2x] if ver == "v3" else None,
            )
            self._cache[ver] = s
            return s

        def validate(self, ver):
            return self.compile(ver).validate(ver)

    dve_ops._SUB_OPCODE_FOR_NAME[NAME] = (
        dve_ops._CUSTOM_DVE_ROW_BASE + len(dve_ops.OPS)
    )
    op = DveOpQ2x()
    dve_ops.OPS.append(op)
    return op


ANT_EGQ2X = _register_egq2x()

N_CORES = 8
B_TOTAL = 10000
SHARD = B_TOTAL // N_CORES  # 1250
C_PER_P = 10  # areas per partition (after padding shard to 1280)
SHARD_PAD = 128 * C_PER_P
AREA = 1024  # 32*32
W = 32
NV = AREA - W  # 992 valid flat positions (rows 0..30)

DEFAULT_VARIANT = "v10-k0-b22-flat-p31-cg-a10-ef8-wip-pip"

F32 = mybir.dt.float32
BF16 = mybir.dt.bfloat16

_NC_CACHE = {}


def _supertiles(shard: int, A: int):
    """Split `shard` areas into supertiles (base, P, a) with a area-slots of
    P partitions each. Area index = base + 128*j + p for slot j, partition p."""
    out = []
    base = 0
    while shard - base >= 128 * A:
        out.append((base, 128, A))
        base += 128 * A
    while shard - base >= 128:
        out.append((base, 128, 1))
        base += 128
    if shard > base:
        out.append((base, shard - base, 1))
        base = shard
    return out


def _build(shard: int, inner_reps: int = 1, A: int = 2, variant: str | None = None) -> bass.Bass:
    if variant is None:
        variant = DEFAULT_VARIANT
    """Per-core SPMD graph: mc [1280,1024] bf16 (edges pre-zeroed, rows
    1250..1279 zero-padded), em [1280,992] bf16 -> out [1280] f32.

    Partition-major layout: area = p*C_PER_P + t, so every DMA is
    contiguous per partition (loads 2-4 KB lines, store one 40 B line).

    Math: loss = sum_k e(t1)*e(t2)*em with e(t) = t*(2-t) = 1-(1-t)^2,
    t1[k]=m[k]*m[k+32], t2[k]=m[k]*m[k+1] over k in [0,992).

    Two-engine split (HW-measured): DVE t1/t2 (same-tensor shifted tt),
    e=1-u (ts), p=e1*e2, w=p*em (tt); ACT squares u=(1-t)^2 and the
    final Copy+accum reduction per area. Lag-pipelined emission."""
    assert shard == SHARD_PAD, shard
    C = C_PER_P
    nc = bacc.Bacc("TRN2", target_bir_lowering=False, debug=False)

    # t9: positions k in [960,992) have t1 = m[k]*m[k+32] = 0 exactly (row 31
    # is zeroed) so e1 = 0 and they contribute nothing; skip loading/computing
    # them. mc only needs k in [0,992) (m[k+32] max index 991).
    # p31: host packs rows at stride 31 (dropping the zeroed col 31): mc is
    # 31x31=961, em is 30x31=930 with col 30 zeroed (kills the row-wrap
    # garbage at j=30). Stream is 930 elems/area, shifts +31/+1.
    if "p31" in variant:
        NVv, MCW, Wv = 930, 961, 31
        MC_DECL, EM_DECL = 961, 930
    else:
        NVv = 960 if "t9" in variant else NV
        MCW = 992 if "t9" in variant else AREA
        Wv = W
        MC_DECL, EM_DECL = AREA, NV

    # f8: inputs stored fp8e4m3 in DRAM, cast to bf16 by the DMA engine
    # during the load (SWDGE dtype-cast path) - halves HBM bytes at zero
    # compute cost. ef8: em only (tighter accuracy margin keeps mc bf16).
    F8 = mybir.dt.float8e4
    mc_f8 = "-f8" in variant
    mc_q8 = "-mq8" in variant
    em_f8 = "-f8" in variant or "-ef8" in variant
    mc_dt = mybir.dt.uint8 if mc_q8 else (F8 if mc_f8 else BF16)
    mc_d = nc.declare_dram_parameter(
        "mc", [shard, MC_DECL], mc_dt, isOutput=False
    )
    em_d = nc.declare_dram_parameter(
        "em", [shard, EM_DECL], F8 if em_f8 else BF16, isOutput=False
    )
    out_d = nc.declare_dram_parameter("out", [shard], F32, isOutput=True)

    if "-a10" in variant:
        A = 10
    elif "-a5" in variant:
        A = 5
    n_super = C // A
    AL = mybir.AluOpType
    AF = mybir.ActivationFunctionType
    mc_v = mc_d.ap().rearrange("(p c) k -> p c k", c=C)
    em_v = em_d.ap().rearrange("(p c) k -> p c k", c=C)

    reuse = "reuse" in variant or "bufs6" in variant
    mid_bufs = 2 if ("b22" in variant or "b23" in variant or "b24" in variant) else (6 if "bufs6" in variant else (3 if ("mix" in variant or "b33" in variant) else (5 if "b53" in variant else 4)))
    lag_c = 1 if "lag1" in variant else (3 if "lag3" in variant else 2)
    ld_bufs = 4 if "b24" in variant else 3 if "b23" in variant else 2 if ("ldb2" in variant or "b22" in variant) else (6 if "ldb6" in variant else (3 if ("b53" in variant or "b33" in variant) else 4))
    # eaN: supertiles s < N compute e12 = 1-u12 on ACT (Copy scale=-1 bias=1)
    # instead of DVE tensor_scalar, shifting ~992c/supertile off DVE.
    ea_n = 0
    if "-ea" in variant and "-eah" not in variant:
        ea_n = int(variant.split("-ea")[1][0])
    with tile.TileContext(nc) as tc:
        with (
            tc.tile_pool(name="ld", bufs=ld_bufs) as ld,
            tc.tile_pool(name="mid", bufs=mid_bufs) as mid,
            tc.tile_pool(name="res", bufs=4) as resp,
            tc.tile_pool(name="stat", bufs=1) as statp,
            tc.tile_pool(name="ps", bufs=2, space="PSUM") as psp,
        ):
          if "mix" in variant and inner_reps:
            stat = {}
            for nm, shp in [("sm", [128, A, AREA]), ("se", [128, A, NV]),
                            ("st", [128, 2, A, NV]), ("su", [128, 2, A, NV]),
                            ("sе12", [128, 2, A, NV]), ("sp", [128, A, NV]),
                            ("sw", [128, A, NV])]:
                t = statp.tile(shp, BF16, tag="stat_" + nm)
                nc.vector.memset(t[:], 0.25)
                stat[nm] = t
          for _rr in range(inner_reps):
            res_t = resp.tile([128, C], F32, tag="res")
            stage_state = {}
            if "mix" in variant:
                for s in range(n_super):
                    c0 = s * A
                    m = ld.tile([128, A, AREA], BF16, tag="m")
                    nc.sync.dma_start(out=m[:], in_=mc_v[:, c0 : c0 + A, :])
                    e = ld.tile([128, A, NV], BF16, tag="e")
                    nc.sync.dma_start(out=e[:], in_=em_v[:, c0 : c0 + A, :])
                    t12 = mid.tile([128, 2, A, NV], BF16, tag="t12")
                    sm = stat["sm"]
                    nc.vector.tensor_tensor(t12[:, 0], sm[:, :, 0:NV], sm[:, :, W:AREA], AL.mult)
                    nc.vector.tensor_tensor(t12[:, 1], sm[:, :, 0:NV], sm[:, :, 1 : 1 + NV], AL.mult)
                    u12 = mid.tile([128, 2, A, NV], BF16, tag="u12")
                    nc.scalar.activation(u12[:], stat["st"][:], AF.Square, bias=1.0, scale=-1.0)
                    e12 = mid.tile([128, 2, A, NV], BF16, tag="e12")
                    nc.vector.tensor_scalar(e12[:], stat["su"][:], -1.0, 1.0, op0=AL.mult, op1=AL.add)
                    p_t = mid.tile([128, A, NV], BF16, tag="p")
                    se12 = stat["sе12"]
                    nc.vector.tensor_tensor(p_t[:], se12[:, 0], se12[:, 1], AL.mult)
                    w = mid.tile([128, A, NV], BF16, tag="w")
                    nc.vector.tensor_tensor(w[:], stat["sp"][:], stat["se"][:], AL.mult)
                    for j in range(A):
                        dum = mid.tile([128, NV], BF16, tag="dum")
                        nc.scalar.activation(
                            dum[:], stat["sw"][:, j], AF.Copy,
                            accum_out=res_t[:, c0 + j : c0 + j + 1],
                        )
                nc.sync.dma_start(
                    out=out_d.ap().rearrange("(p c) -> p c", c=C), in_=res_t[:]
                )
                continue

            def stage_a(s):
                c0 = s * A
                if "-big1" in variant:
                    # one whole-pass DMA per tensor: bigger transfers, 1/5th
                    # the fixed DMA costs; chained reps still double-buffer
                    # through the ld pool
                    if s == 0:
                        mb = ld.tile([128, C, MCW], BF16, tag="m")
                        (nc.gpsimd if mc_f8 else nc.sync).dma_start(
                            out=mb[:], in_=mc_v[:, :, 0:MCW]
                        )
                        eb = ld.tile([128, C, NVv], BF16, tag="e")
                        (nc.gpsimd if em_f8 else nc.sync).dma_start(
                            out=eb[:], in_=em_v[:, :, 0:NVv]
                        )
                        stage_state["mb"] = mb
                        stage_state["eb"] = eb
                    mb = stage_state["mb"]
                    eb = stage_state["eb"]
                    e_ap = eb[:, c0 : c0 + A]
                    m0 = mb[:, c0 : c0 + A, 0:NVv]
                    mW = mb[:, c0 : c0 + A, Wv : Wv + NVv]
                    m1 = mb[:, c0 : c0 + A, 1 : 1 + NVv]
                    t12 = mid.tile([128, 2, A, NVv], BF16, tag="t12")
                    nc.vector.tensor_tensor(t12[:, 0], m0, mW, AL.mult)
                    nc.vector.tensor_tensor(t12[:, 1], m0, m1, AL.mult)
                    stage_state[s] = (e_ap, t12, None)
                    return
                m = ld.tile([128, A, MCW], BF16, tag="m")
                # emul: no em tile at all - the em load is a CCE mult into p
                # during stage_c (see below)
                e = None if "-emul" in variant else ld.tile(
                    [128, A, NVv], BF16, tag="e"
                )
                if variant == "tinydma":
                    nc.sync.dma_start(out=m[:, :, 0:16], in_=mc_v[:, c0 : c0 + A, 0:16])
                    nc.sync.dma_start(out=e[:, :, 0:16], in_=em_v[:, c0 : c0 + A, 0:16])
                else:
                    mc_eng = nc.gpsimd if (mc_f8 or mc_q8) else (
                        nc.scalar if "-mcs" in variant else nc.sync
                    )
                    mc_eng.dma_start(out=m[:], in_=mc_v[:, c0 : c0 + A, 0:MCW])
                    if e is not None:
                        (nc.gpsimd if em_f8 else nc.sync).dma_start(
                            out=e[:], in_=em_v[:, c0 : c0 + A, 0:NVv]
                        )
                if variant == "dmaonly" or "-xcomp" in variant:
                    stage_state[s] = (e, None, None)
                    return
                if "-cg" in variant:
                    # t-products fused into the 2X custom e() op in stage_c
                    stage_state[s] = (e, m, None)
                    return
                if "v10" in variant or "v13" in variant:
                    t12 = mid.tile([128, 2, A, NVv], BF16, tag="t12")
                    nc.vector.tensor_tensor(
                        t12[:, 0], m[:, :, 0:NVv], m[:, :, Wv : Wv + NVv], AL.mult
                    )
                    if "-xal" in variant:  # timing probe: aligned in1 (wrong math)
                        nc.vector.tensor_tensor(
                            t12[:, 1], m[:, :, 0:NVv], m[:, :, 0:NVv], AL.mult
                        )
                    elif "-xt2" not in variant:  # timing ablation: drop t2 op
                        nc.vector.tensor_tensor(
                            t12[:, 1], m[:, :, 0:NVv], m[:, :, 1 : 1 + NVv], AL.mult
                        )
                    stage_state[s] = (e, t12, None)
                    return
                t1 = mid.tile([128, A, NV], BF16, tag="t1")
                nc.vector.tensor_tensor(t1[:], m[:, :, 0:NV], m[:, :, W:AREA], AL.mult)
                t2 = mid.tile([128, A, NV], BF16, tag="t2")
                nc.vector.tensor_tensor(t2[:], m[:, :, 0:NV], m[:, :, 1 : 1 + NV], AL.mult)
                stage_state[s] = (e, t1, t2)

            def stage_b(s):
                if variant == "dmaonly" or "-xcomp" in variant:
                    return
                e, t1, t2 = stage_state[s]
                if "-cg" in variant:
                    return
                if "formB" in variant and s == 2:
                    t12 = t1
                    g12 = mid.tile([128, 2, A, NV], BF16, tag="u12")
                    nc.vector.tensor_scalar(
                        g12[:], t12[:], -1.0, 2.0, op0=AL.mult, op1=AL.add
                    )
                    stage_state[s] = (e, t12, g12)
                    return
                if "v10" in variant or "v13" in variant:
                    t12 = t1
                    if "-cp" in variant:
                        if "-eah" in variant:
                            # ACT-offload: for hn of the A area slots, e12 is
                            # computed on ACT (Square then 1-u Copy), freeing
                            # the DVE EE2 op for those slots; DVE later does
                            # just p = e1*e2 (2x tt) for them.
                            hn = int(variant.split("-eah")[1][0])
                            u12 = mid.tile([128, 2, hn, NVv], BF16, tag="u12")
                            nc.scalar.activation(
                                u12[:], t12[:, :, 0:hn], AF.Square,
                                bias=1.0, scale=-1.0,
                            )
                            # e12 = 1-u12 in place (second ACT pass, same tile)
                            nc.scalar.activation(
                                u12[:], u12[:], AF.Copy, bias=1.0, scale=-1.0
                            )
                            stage_state[s] = (e, t12, u12)
                            return
                        # fused custom op computes p straight from t12 in
                        # stage_c; no u12/e12 tiles needed at all
                        stage_state[s] = (e, t12, None)
                        return
                    u12 = mid.tile([128, 2, A, NVv], BF16, tag="u12")
                    e12 = mid.tile([128, 2, A, NVv], BF16, tag="e12")
                    if "flat" in variant:
                        # flat 2D APs so the elementwise map can hit the
                        # fastest DVE perf mode (multi-dim APs cap it)
                        t12f = t12.rearrange("p x a k -> p (x a k)")
                        u12f = u12.rearrange("p x a k -> p (x a k)")
                        e12f = e12.rearrange("p x a k -> p (x a k)")
                        nc.scalar.activation(
                            u12f[:], t12f[:], AF.Square, bias=1.0, scale=-1.0
                        )
                        if s < ea_n:
                            nc.scalar.activation(
                                e12f[:], u12f[:], AF.Copy, bias=1.0, scale=-1.0
                            )
                        else:
                            nc.vector.tensor_scalar(
                                e12f[:], u12f[:], -1.0, 1.0, op0=AL.mult, op1=AL.add
                            )
                        stage_state[s] = (e, e12, None)
                        return
                    nc.scalar.activation(u12[:], t12[:], AF.Square, bias=1.0, scale=-1.0)
                    nc.vector.tensor_scalar(
                        e12[:], u12[:], -1.0, 1.0, op0=AL.mult, op1=AL.add
                    )
                    stage_state[s] = (e, e12, None)
                    return
                u1 = mid.tile([128, A, NV], BF16, tag="u1")
                nc.scalar.activation(u1[:], t1[:], AF.Square, bias=1.0, scale=-1.0)
                u2 = mid.tile([128, A, NV], BF16, tag="u2")
                nc.scalar.activation(u2[:], t2[:], AF.Square, bias=1.0, scale=-1.0)
                if "v9" in variant:
                    # e1,e2 share one tile so p = e1*e2 hits the DVE
                    # single-source fastpath
                    e12 = mid.tile([128, 2, A, NV], BF16, tag="e12")
                    nc.vector.tensor_scalar(
                        e12[:, 0], u1[:], -1.0, 1.0, op0=AL.mult, op1=AL.add
                    )
                    nc.vector.tensor_scalar(
                        e12[:, 1], u2[:], -1.0, 1.0, op0=AL.mult, op1=AL.add
                    )
                    stage_state[s] = (e, e12, None)
                    return
                e1 = mid.tile([128, A, NV], BF16, tag="t1" if reuse else "e1")
                nc.vector.tensor_scalar(e1[:], u1[:], -1.0, 1.0, op0=AL.mult, op1=AL.add)
                e2 = mid.tile([128, A, NV], BF16, tag="t2" if reuse else "e2")
                nc.vector.tensor_scalar(e2[:], u2[:], -1.0, 1.0, op0=AL.mult, op1=AL.add)
                stage_state[s] = (e, e1, e2)

            def stage_c(s):
                c0 = s * A
                e, e1, e2 = stage_state.pop(s)
                if variant == "dmaonly" or "-xcomp" in variant:
                    return
                if "formB" in variant and s == 2:
                    t12, g12 = e1, e2
                    v = mid.tile([128, A, NV], BF16, tag="p")
                    nc.vector.tensor_tensor(v[:], t12[:, 0], t12[:, 1], AL.mult)
                    d = mid.tile([128, A, NV], BF16, tag="d")
                    nc.vector.tensor_tensor(d[:], g12[:, 0], g12[:, 1], AL.mult)
                    p_t = mid.tile([128, A, NV], BF16, tag="p2")
                    nc.vector.tensor_tensor(p_t[:], v[:], d[:], AL.mult)
                    w = mid.tile([128, A, NV], BF16, tag="w")
                    nc.vector.tensor_tensor(w[:], p_t[:], e[:], AL.mult)
                    for j in range(A):
                        dum = mid.tile([128, NV], BF16, tag="dum")
                        nc.scalar.activation(
                            dum[:], w[:, j], AF.Copy,
                            accum_out=res_t[:, c0 + j : c0 + j + 1],
                        )
                    return
                if "v13" in variant:
                    # total-work-lean: per-slot stt fuses w=p*em with the
                    # f32 accumulate; no w tile, no ACT reductions
                    e12 = e1
                    p_t = mid.tile([128, A, NV], BF16, tag="p")
                    nc.vector.tensor_tensor(p_t[:], e12[:, 0], e12[:, 1], AL.mult)
                    for j in range(A):
                        qd = mid.tile([128, NV], BF16, tag="dum")
                        nc.vector.scalar_tensor_tensor(
                            qd[:], p_t[:, j], 1.0, e[:, j],
                            op0=AL.mult, op1=AL.mult,
                            accum_out=res_t[:, c0 + j : c0 + j + 1],
                        )
                    return
                if "v9" in variant or "v10" in variant:
                    e12 = e1
                    p_t = None if "-pip" in variant else mid.tile(
                        [128, A, NVv], BF16, tag="p"
                    )
                    if "-cg" in variant:
                        # g_i = e(t_i) straight from m via the hand-2X custom
                        # op (2 elems/cycle); p = g1*g2 on the stock 2x tt.
                        m = e1
                        if "-psep" in variant:
                            g1_t = mid.tile([128, A, NVv], BF16, tag="t12")
                            g2_t = mid.tile([128, A, NVv], BF16, tag="g2")
                            g_lo, g_hi = g1_t[:], g2_t[:]
                        else:
                            g12 = mid.tile([128, 2, A, NVv], BF16, tag="t12")
                            g_lo, g_hi = g12[:, 0], g12[:, 1]
                        if "-mq8" in variant:
                            gop, gs0, gs1 = ANT_EGQ2X, 2.0, 1.0 / (255.0 * 255.0)
                            kw = {"s1": gs1}
                        else:
                            gop, gs0, kw = ANT_EG2X, 2.0, {}
                        b1 = nc.vector._custom_dve(
                            gop, out=g_lo, in0=m[:, :, 0:NVv],
                            in1=m[:, :, Wv : Wv + NVv], s0=gs0, **kw,
                        )
                        b1.ins.perf_max = 1
                        if "-xg2" not in variant:  # timing ablation
                            b2 = nc.vector._custom_dve(
                                gop, out=g_hi, in0=m[:, :, 0:NVv],
                                in1=m[:, :, 1 : 1 + NVv], s0=gs0, **kw,
                            )
                            b2.ins.perf_max = 1
                        if "-xp" in variant:  # timing ablation: skip p op
                            p_t = g_lo
                        elif "-pip" in variant:
                            # p = g1*g2 in place into g12[:,0]: no p tile at
                            # all, freeing SBUF for deeper ld prefetch (b23)
                            g12v = g12.rearrange("p x a k -> p x (a k)")
                            nc.vector.tensor_tensor(
                                g12v[:, 0], g12v[:, 0], g12v[:, 1], AL.mult
                            )
                            p_t = g12[:, 0]
                        elif "-psep" in variant:
                            nc.vector.tensor_tensor(p_t[:], g1_t[:], g2_t[:], AL.mult)
                        else:
                            g12v = g12.rearrange("p x a k -> p x (a k)")
                            p_f = p_t.rearrange("p a k -> p (a k)")
                            nc.vector.tensor_tensor(
                                p_f[:], g12v[:, 0], g12v[:, 1], AL.mult
                            )
                    elif "-cp" in variant:
                        # p = e(t1)*e(t2) in one fused DVE op (1x rate =
                        # same DVE cycles as the e12 ts + p tt it replaces)
                        # -> ACT's Square over u12 disappears entirely
                        t12v = e12.rearrange("p x a k -> p x (a k)")
                        p_f = p_t.rearrange("p a k -> p (a k)")
                        if e2 is not None:  # -eah: first hn slots via ACT e12
                            hn = e2.shape[2]
                            nc.vector.tensor_tensor(
                                p_t[:, 0:hn], e2[:, 0], e2[:, 1], AL.mult
                            )
                            nc.vector._custom_dve(
                                ANT_EE2,
                                out=p_t[:, hn:A],
                                in0=e12[:, 0, hn:A],
                                in1=e12[:, 1, hn:A],
                            )
                        elif "-xee2" in variant:  # timing ablation: stock tt
                            nc.vector.tensor_tensor(
                                p_f[:], t12v[:, 0], t12v[:, 1], AL.mult
                            )
                        else:
                            nc.vector._custom_dve(
                                ANT_EE2, out=p_f[:], in0=t12v[:, 0], in1=t12v[:, 1]
                            )
                        if "-caw" in variant:
                            # w = p*em fused with the per-area reduction in
                            # one DVE op each: ACT leaves the steady state
                            for j in range(A):
                                dum = mid.tile([128, NVv], BF16, tag="dum")
                                nc.vector.affine_mul_reduce(
                                    out=dum[:],
                                    accum_out=res_t[:, c0 + j : c0 + j + 1],
                                    in0=p_t[:, j],
                                    in1=e[:, j],
                                    scale=1.0,
                                    bias=0.0,
                                )
                            return
                    else:
                        nc.vector.tensor_tensor(p_t[:], e12[:, 0], e12[:, 1], AL.mult)
                    if "-emul" in variant:
                        # w = p*em computed by the SWDGE CCE during the em
                        # load itself: DRAM em (fp8, cast) multiplies into
                        # the p tile. No em SBUF tile, no DVE w op.
                        nc.gpsimd.dma_start(
                            out=p_t[:], in_=em_v[:, c0 : c0 + A, 0:NVv],
                            accum_op=AL.mult,
                        )
                        w = p_t
                    elif "-w7" in variant:
                        # w computed by the DMA compute engine (CCE mult),
                        # in-place into the em tile: frees ~4800c of DVE
                        nc.gpsimd.dma_start(out=e[:], in_=p_t[:], accum_op=AL.mult)
                        w = e
                    elif "-xw" in variant:  # timing ablation: drop w op
                        w = p_t
                    elif "-wip" in variant:
                        # w = p*em in place into the p tile (one less tile)
                        nc.vector.tensor_tensor(p_t[:], p_t[:], e[:], AL.mult)
                        w = p_t
                    else:
                        w = mid.tile([128, A, NVv], BF16, tag="w")
                        nc.vector.tensor_tensor(w[:], p_t[:], e[:], AL.mult)
                    if "-xred" in variant:  # timing ablation: no reductions
                        return
                    if "fold1" in variant:
                        # halve reduction length with one same-tile tt-add
                        wh = mid.tile([128, A, NV // 2], BF16, tag="wh")
                        nc.vector.tensor_tensor(
                            wh[:], w[:, :, 0 : NV // 2], w[:, :, NV // 2 : NV], AL.add
                        )
                        for j in range(A):
                            dum = mid.tile([128, NV // 2], BF16, tag="dumh")
                            nc.scalar.activation(
                                dum[:], wh[:, j], AF.Copy,
                                accum_out=res_t[:, c0 + j : c0 + j + 1],
                            )
                        return
                    for j in range(A):
                        if "k0" not in variant and s in (1, 3) and j == 0:
                            qd = mid.tile([128, NVv], BF16, tag="dum")
                            nc.vector.tensor_scalar(
                                qd[:], w[:, j], 1.0, None, op0=AL.mult, op1=AL.add,
                                accum_out=res_t[:, c0 + j : c0 + j + 1],
                            )
                        elif "pdum" in variant:
                            # acc dummy write goes to PSUM: off the SBUF ports
                            dum = psp.tile([128, NVv], F32, tag="pdum")
                            nc.scalar.activation(
                                dum[:], w[:, j], AF.Copy,
                                accum_out=res_t[:, c0 + j : c0 + j + 1],
                            )
                        elif "f8dum" in variant:
                            dum = mid.tile([128, NVv], mybir.dt.float8e4, tag="dum8f")
                            nc.scalar.activation(
                                dum[:], w[:, j], AF.Copy,
                                accum_out=res_t[:, c0 + j : c0 + j + 1],
                            )
                        else:
                            dum = mid.tile([128, NVv], BF16, tag="dum")
                            nc.scalar.activation(
                                dum[:], w[:, j], AF.Copy,
                                accum_out=res_t[:, c0 + j : c0 + j + 1],
                            )
                    return
                p_t = mid.tile([128, A, NV], BF16, tag="u1" if reuse else "p")
                nc.vector.tensor_tensor(p_t[:], e1[:], e2[:], AL.mult)
                if "v8" in variant:
                    # w = p*em on DVE, then DMA-CCE tree-fold (add) 992->124
                    # so the per-area reductions touch only 124 elements
                    w = mid.tile([128, A, NV], BF16, tag="u2" if reuse else "w")
                    nc.vector.tensor_tensor(w[:], p_t[:], e[:], AL.mult)
                    nc.gpsimd.dma_start(
                        out=w[:, :, 0:496], in_=w[:, :, 496:992], accum_op=AL.add
                    )
                    nc.gpsimd.dma_start(
                        out=w[:, :, 0:248], in_=w[:, :, 248:496], accum_op=AL.add
                    )
                    nc.gpsimd.dma_start(
                        out=w[:, :, 0:124], in_=w[:, :, 124:248], accum_op=AL.add
                    )
                    for j in range(A):
                        dum = mid.tile([128, 124], BF16, tag="dum8")
                        nc.scalar.activation(
                            dum[:], w[:, j, 0:124], AF.Copy,
                            accum_out=res_t[:, c0 + j : c0 + j + 1],
                        )
                    return
                if "v7" in variant:
                    # w = p*em computed by the DMA compute engine, in-place
                    # into the em tile; reductions split DVE/ACT per slot
                    nc.gpsimd.dma_start(out=e[:], in_=p_t[:], accum_op=AL.mult)
                    for j in range(A):
                        if j % 2 == 0:
                            qd = mid.tile([128, NV], BF16, tag="dum")
                            nc.vector.tensor_scalar(
                                qd[:], e[:, j], 1.0, None, op0=AL.mult,
                                op1=AL.add,
                                accum_out=res_t[:, c0 + j : c0 + j + 1],
                            )
                        else:
                            dum = mid.tile([128, NV], BF16, tag="dum")
                            nc.scalar.activation(
                                dum[:], e[:, j], AF.Copy,
                                accum_out=res_t[:, c0 + j : c0 + j + 1],
                            )
                    return
                w = mid.tile([128, A, NV], BF16, tag="u2" if reuse else "w")
                if "gw" in variant:
                    nc.gpsimd.tensor_tensor(w[:], p_t[:], e[:], AL.mult)
                else:
                    nc.vector.tensor_tensor(w[:], p_t[:], e[:], AL.mult)
                for j in range(A):
                    dum = mid.tile([128, NV], BF16, tag="dum")
                    nc.scalar.activation(
                        dum[:], w[:, j], AF.Copy,
                        accum_out=res_t[:, c0 + j : c0 + j + 1],
                    )

            for s in range(n_super + lag_c):
                if s < n_super:
                    stage_a(s)
                if 1 <= s < n_super + 1:
                    stage_b(s - 1)
                if s >= lag_c:
                    stage_c(s - lag_c)

            if variant == "dmaonly" or "-xred" in variant or "-xcomp" in variant:
                nc.gpsimd.memset(res_t[:], 0.0)
            if variant != "lastout" or _rr == inner_reps - 1:
                nc.sync.dma_start(
                    out=out_d.ap().rearrange("(p c) -> p c", c=C), in_=res_t[:]
                )

    nc.compile()
    return nc

class _Exec:
    """One-time-jitted SPMD executor for a prebuilt Bass graph.

    Vendored from bass2jax.run_bass_via_pjrt so repeated calls reuse the
    compiled executable (run_bass_via_pjrt re-jits per invocation)."""

    def __init__(self, nc: bass.Bass, n_cores: int):
        bass2jax.install_neuronx_cc_hook()
        assert nc.dbg_addr is None or not nc.dbg_callbacks
        partition_name = (
            nc.partition_id_tensor.name if nc.partition_id_tensor else None
        )
        in_names, out_names, out_avals = [], [], []
        for alloc in nc.m.functions[0].allocations:
            if not isinstance(alloc, mybir.MemoryLocationSet):
                continue
            name = alloc.memorylocations[0].name
            if alloc.kind == "ExternalInput":
                if name != partition_name and name != getattr(nc.dbg_addr, "name", None):
                    in_names.append(name)
            elif alloc.kind == "ExternalOutput":
                shape = tuple(alloc.tensor_shape)
                dtype = mybir.dt.np(alloc.dtype)
                out_names.append(name)
                out_avals.append(jax.core.ShapedArray(shape, dtype))
        self.in_names = list(in_names)
        self.out_names = out_names
        self.out_avals = out_avals
        self.n_cores = n_cores
        n_params = len(in_names)
        n_outs = len(out_avals)

        all_in_names = list(in_names) + list(out_names)
        if nc.dbg_addr is not None:
            all_in_names.append(nc.dbg_addr.name)
        if partition_name is not None:
            all_in_names.append(partition_name)
        self._has_dbg = nc.dbg_addr is not None

        def jnp_zeros_dbg():
            import jax.numpy as jnp

            return jnp.zeros((1, 2), np.uint32)

        def _call_once(ins, outs):
            operands = list(ins) + list(outs)
            if self._has_dbg:
                operands.append(jnp_zeros_dbg())
            if partition_name is not None:
                operands.append(bass2jax.partition_id_tensor())
            return tuple(
                bass2jax._bass_exec_p.bind(
                    *operands,
                    out_avals=tuple(out_avals),
                    in_names=tuple(all_in_names),
                    out_names=tuple(out_names),
                    lowering_input_output_aliases=(),
                    sim_require_finite=True,
                    sim_require_nnan=True,
                    nc=nc,
                )
            )

        self._call_once = _call_once

        def _body(*args):
            return _call_once(args[:n_params], args[n_params:])

        devices = jax.devices()[:n_cores]
        assert len(devices) == n_cores
        self.mesh = Mesh(np.asarray(devices), ("core",))
        in_specs = (PartitionSpec("core"),) * (n_params + n_outs)
        out_specs = (PartitionSpec("core"),) * n_outs
        donate = tuple(range(n_params, n_params + n_outs))
        self._fn = jax.jit(
            shard_map(
                _body,
                mesh=self.mesh,
                in_specs=in_specs,
                out_specs=out_specs,
                check_rep=False,
            ),
            donate_argnums=donate,
            keep_unused=True,
        )
        self.sharding = NamedSharding(self.mesh, PartitionSpec("core"))
        self._n_params = n_params
        self._n_outs = n_outs
        self._in_specs = in_specs
        self._chain_cache = {}

    def chain_fn(self, n: int):
        """Jitted fn executing the NEFF n times, serialized via the out bufs."""
        if n not in self._chain_cache:
            def _chain_body(*args):
                ins = args[: self._n_params]
                outs = tuple(args[self._n_params :])
                for _ in range(n):
                    outs = self._call_once(ins, outs)
                return outs

            donate = tuple(range(self._n_params, self._n_params + self._n_outs))
            self._chain_cache[n] = jax.jit(
                shard_map(
                    _chain_body,
                    mesh=self.mesh,
                    in_specs=self._in_specs,
                    out_specs=(PartitionSpec("core"),) * self._n_outs,
                    check_rep=False,
                ),
                donate_argnums=donate,
                keep_unused=True,
            )
        return self._chain_cache[n]

    def time_chain(self, concat_in_dev, n: int, reps: int = 10):
        import time

        fn = self.chain_fn(n)
        for _ in range(2):
            jax.block_until_ready(fn(*concat_in_dev, *self.fresh_zeros()))
        times = []
        for _ in range(reps):
            zeros = self.fresh_zeros()
            jax.block_until_ready(zeros)
            t0 = time.perf_counter()
            jax.block_until_ready(fn(*concat_in_dev, *zeros))
            times.append(time.perf_counter() - t0)
        return min(times)

    def concat_inputs(self, in_maps):
        return [
            np.concatenate([np.asarray(m[name]) for m in in_maps], axis=0)
            for name in self.in_names
        ]

    def fresh_zeros(self):
        return [
            jax.device_put(
                np.zeros((self.n_cores * a.shape[0], *a.shape[1:]), a.dtype),
                self.sharding,
            )
            for a in self.out_avals
        ]

    def __call__(self, concat_in):
        out_arrs = self._fn(*concat_in, *self.fresh_zeros())
        return [np.asarray(o) for o in out_arrs]


_EXEC_CACHE = {}


def _get_exec(shard: int, inner_reps: int = 1, variant: str | None = None) -> _Exec:
    if variant is None:
        variant = DEFAULT_VARIANT
    key = (shard, inner_reps, variant)
    if key not in _EXEC_CACHE:
        _EXEC_CACHE[key] = _Exec(_build(shard, inner_reps, variant=variant), N_CORES)
    return _EXEC_CACHE[key]


def _benchmark(mask_combined, edge_map, mask_index=1, inner_reps=257, reps=40, variant=None):
    """Measure steady-state per-pass device time by comparing a 1-rep NEFF
    against an inner_reps-rep NEFF. Calls are interleaved pairwise and the
    median difference is used, cancelling the multi-ms drift of the ~80 ms
    axon dispatch overhead."""
    import time

    full, shard = _prep_inputs(mask_combined, edge_map, mask_index, variant)
    ex1 = _get_exec(SHARD_PAD, 1, variant=variant)
    exR = _get_exec(SHARD_PAD, inner_reps, variant=variant)
    dev_in = [jax.device_put(full[name], ex1.sharding) for name in ex1.in_names]
    fn1 = ex1.chain_fn(1)
    fnR = exR.chain_fn(1)

    def one(ex, fn):
        z = ex.fresh_zeros()
        jax.block_until_ready(z)
        t0 = time.perf_counter()
        jax.block_until_ready(fn(*dev_in, *z))
        return time.perf_counter() - t0

    for _ in range(3):
        one(ex1, fn1)
        one(exR, fnR)
    diffs = []
    t1s = []
    for _ in range(reps):
        a = one(ex1, fn1)
        b = one(exR, fnR)
        t1s.append(a)
        diffs.append(b - a)
    diffs = np.array(diffs)
    loop_ns = float(np.median(diffs)) / (inner_reps - 1) * 1e9
    return {
        "dispatch_1rep_ns": float(np.min(t1s)) * 1e9,
        "diff_med_ns": float(np.median(diffs)) * 1e9,
        "diff_p25_ns": float(np.percentile(diffs, 25)) * 1e9,
        "diff_p75_ns": float(np.percentile(diffs, 75)) * 1e9,
        "loop_ns": loop_ns,
    }


def _prep_inputs(mask_combined, edge_map, mask_index, variant=None):
    import ml_dtypes

    if variant is None:
        variant = DEFAULT_VARIANT
    bf16 = ml_dtypes.bfloat16
    idx = int(np.asarray(mask_index))
    # convert up front: jax-array inputs would otherwise dispatch jax ops
    # (potentially on the neuron backend) during host-side prep
    mask_combined = np.asarray(mask_combined)
    edge_map = np.asarray(edge_map)
    B = mask_combined.shape[0]
    assert B % N_CORES == 0, B
    shard = B // N_CORES
    assert shard <= SHARD_PAD
    mc = np.asarray(mask_combined[..., idx], dtype=np.float32).astype(bf16)
    mc[:, :, -1] = 0  # reference zeroes last col/row of the selected mask
    mc[:, -1, :] = 0
    if "p31" in variant:
        # dense 31-stride packing: mc rows 0..30 cols 0..30 (961), em rows
        # 0..29 at 31-wide with col 30 zeroed (930). Same math, 3% fewer
        # elements in every stream.
        mc = np.ascontiguousarray(mc[:, 0:31, 0:31]).reshape(B, 961)
        em2 = np.asarray(edge_map, dtype=np.float32)[..., 0]
        em = np.zeros((B, 30, 31), bf16)
        em[:, :, 0:30] = em2[:, 0:30, 0:30].astype(bf16)
        em = em.reshape(B, 930)
    else:
        mc = mc.reshape(B, AREA)
        em = (
            np.asarray(edge_map, dtype=np.float32)[..., 0]
            .reshape(B, AREA)[:, :NV]
            .astype(bf16)
        )
    if "-mq8" in variant:
        # uint8 fixed-point: m ~ q/255, round to nearest. RMS quantization
        # error ~1.1e-3 absolute, ~20x better than fp8e4m3 at 1 byte/elem.
        mc = np.rint(mc.astype(np.float32) * 255.0).clip(0, 255).astype(np.uint8)
    elif "-f8" in variant:
        mc = mc.astype(ml_dtypes.float8_e4m3)
    if "-f8" in variant or "-ef8" in variant:
        em = em.astype(ml_dtypes.float8_e4m3)

    # pad each core's shard to SHARD_PAD rows of zeros (zero areas -> zero loss)
    def pad(x):
        x = x.reshape(N_CORES, shard, x.shape[-1])
        out = np.zeros((N_CORES, SHARD_PAD, x.shape[-1]), x.dtype)
        out[:, :shard] = x
        return out.reshape(N_CORES * SHARD_PAD, x.shape[-1])

    return {"mc": pad(mc), "em": pad(em)}, shard


def _run(resized_image=None, mask_combined=None, edge_map=None, mask_index=1, variant=None, **_):
    full, shard = _prep_inputs(mask_combined, edge_map, mask_index, variant)
    ex = _get_exec(SHARD_PAD, variant=variant)
    concat_in = [full[name] for name in ex.in_names]
    outs = ex(concat_in)
    out = outs[ex.out_names.index("out")].reshape(N_CORES, SHARD_PAD)[:, :shard]
    return out.reshape(-1).astype(np.float32, copy=False), ex


# fallback chain: previous proven custom-op config (ANT_EE2, no hand-2X),
# then a stock-op-only variant (no custom DVE, no fp8) as final insurance
FALLBACK_VARIANTS = [
    "v10-k0-b22-flat-p31-cg-a10-ef8-wip",
    "v10-k0-b33-flat-p31-cg-a5-ef8",
    "v10-k0-b33-flat-p31-cp-a5-ef8",
    "v10-k0-b53-flat-p31",
]
FALLBACK_VARIANT = FALLBACK_VARIANTS[-1]


def kernel(**inputs) -> np.ndarray:
    try:
        out, _ = _run(**inputs)
        return out
    except Exception:
        pass
    for v in FALLBACK_VARIANTS:
        try:
            out, _ = _run(variant=v, **inputs)
            return out
        except Exception:
            if v == FALLBACK_VARIANTS[-1]:
                raise
    raise RuntimeError("unreachable")


def _time_reps(resized_image=None, mask_combined=None, edge_map=None, mask_index=1, reps=30, **_):
    import time

    full, shard = _prep_inputs(mask_combined, edge_map, mask_index)
    ex = _get_exec(shard)
    concat_in = [
        jax.device_put(full[name], ex.sharding) for name in ex.in_names
    ]
    for _i in range(3):
        jax.block_until_ready(ex._fn(*concat_in, *ex.fresh_zeros()))
    times = []
    for _i in range(reps):
        zeros = ex.fresh_zeros()
        jax.block_until_ready(zeros)
        t0 = time.perf_counter()
        jax.block_until_ready(ex._fn(*concat_in, *zeros))
        times.append(time.perf_counter() - t0)
    return times


def _build_null() -> bass.Bass:
    nc = bacc.Bacc("TRN2", target_bir_lowering=False, debug=False)
    x_d = nc.declare_dram_parameter("x", [128, 8], F32, isOutput=False)
    y_d = nc.declare_dram_parameter("y", [128, 8], F32, isOutput=True)
    with tile.TileContext(nc) as tc:
        with tc.tile_pool(name="p", bufs=1) as pool:
            t = pool.tile([128, 8], F32)
            nc.sync.dma_start(out=t[:], in_=x_d.ap()[:])
            nc.sync.dma_start(out=y_d.ap()[:], in_=t[:])
    nc.compile()
    return nc


def _time_null(reps=30):
    import time

    if "null" not in _EXEC_CACHE:
        _EXEC_CACHE["null"] = _Exec(_build_null(), N_CORES)
    ex = _EXEC_CACHE["null"]
    x = np.zeros((N_CORES * 128, 8), np.float32)
    concat_in = [jax.device_put(x, ex.sharding)]
    for _i in range(3):
        jax.block_until_ready(ex._fn(*concat_in, *ex.fresh_zeros()))
    times = []
    for _i in range(reps):
        zeros = ex.fresh_zeros()
        jax.block_until_ready(zeros)
        t0 = time.perf_counter()
        jax.block_until_ready(ex._fn(*concat_in, *zeros))
        times.append(time.perf_counter() - t0)
    return times

